# revision 1
# baseline (speedup 1.0000x reference)
"""HSTU attention (B=2, L=2048, D=1024, H=16) on 8 TRN2 NeuronCores.

Sharding: batch (2) x head-group (4 heads, 256 features) -> 8 cores.
Each core computes, for its batch b and its 4 heads:
  QT/KT/UT = (x_b @ W.T).T slices in [e, l] layout, V in [l, e] layout,
  S^T = K^T.T-style scores in [j, i] layout (j = keys on partitions),
  expS with the HSTU hybrid mask folded in (per-partition bias for
  off-diagonal tiles, host-precomputed additive mask for the 16 diagonal
  tiles), O^T = V_aug.T @ expS with a ones column giving the softmax row
  sums, gating with U and 1/rowsum (broadcast via ones outer-product
  matmuls), then the row-sharded W_o partial projection.
Host sums the 4 partial outputs per batch.

All matmuls run in float32r (fp32 rounded to 11-bit mantissa, full PE
rate at N>=256); operands are produced by DVE/ACT ops so walrus accepts
them as fp32r-rounded.
"""

import sys

for _p in ("/opt/trn_rl_repo", "/root/.axon_site/_ro/trn_rl_repo"):
    if _p not in sys.path:
        sys.path.insert(0, _p)

import numpy as np

import concourse.bass as bass  # noqa: F401  (bass types used via tile/bacc)
import concourse.mybir as mybir
import concourse.tile as tile
from concourse import bacc
from concourse.bass_utils import run_bass_kernel_spmd

F32 = mybir.dt.float32
F32R = mybir.dt.float32r
EXP = mybir.ActivationFunctionType.Exp

B, L, D, H = 2, 2048, 1024, 16
DK = D // H          # 64
HPC = 4              # heads per core
E = HPC * DK         # 256 features per core
NJC = L // 128       # 16 j-chunks (keys, 128-partition tiles)
NIC = L // 512       # 4 i-chunks (queries, 512 free)
NDC = D // 128       # 8 d-chunks (contraction for projections)
NEG = -10000.0

_cache = {}


def _build():
    nc = bacc.Bacc("TRN2", target_bir_lowering=False, debug=False)

    xt = nc.dram_tensor("xt", [D, L], F32, kind="ExternalInput").ap()
    wq = nc.dram_tensor("wq", [D, E], F32, kind="ExternalInput").ap()
    wk = nc.dram_tensor("wk", [D, E], F32, kind="ExternalInput").ap()
    wv = nc.dram_tensor("wv", [D, E], F32, kind="ExternalInput").ap()
    wu = nc.dram_tensor("wu", [D, E], F32, kind="ExternalInput").ap()
    wo = nc.dram_tensor("wo", [E, D], F32, kind="ExternalInput").ap()
    biasab = nc.dram_tensor("biasab", [128, NJC], F32, kind="ExternalInput").ap()
    biasbl = nc.dram_tensor("biasbl", [128, NJC], F32, kind="ExternalInput").ap()
    dmask = nc.dram_tensor("dmask", [NJC, 128, 512], F32, kind="ExternalInput").ap()
    out = nc.dram_tensor("out", [L, D], F32, kind="ExternalOutput").ap()

    with tile.TileContext(nc) as tc:
        with tc.tile_pool(name="persist", bufs=1) as persist:
            qt = [persist.tile([128, L], F32R, tag=f"qt{i}", name=f"qt{i}") for i in range(2)]
            kt = [persist.tile([128, L], F32R, tag=f"kt{i}", name=f"kt{i}") for i in range(2)]
            ut = [persist.tile([128, L], F32, tag=f"ut{i}", name=f"ut{i}") for i in range(2)]
            g = [persist.tile([128, L], F32R, tag=f"g{i}", name=f"g{i}") for i in range(2)]
            # v layout per jc: [V_h0 | ones | zeros63 | V_h1] ++ same for h2/h3
            v = persist.tile([128, NJC, 384], F32R, tag="v")
            wo_r = [persist.tile([128, D], F32R, tag=f"wor{i}", name=f"wor{i}") for i in range(2)]
            bias_ab_t = persist.tile([128, NJC], F32, tag="bab")
            bias_bl_t = persist.tile([128, NJC], F32, tag="bbl")
            onesf = persist.tile([128, 128], F32, tag="onesf")
            zerof = persist.tile([128, 63], F32, tag="zerof")
            ones_r = persist.tile([128, 128], F32R, tag="onesr")

            nc.sync.dma_start(out=bias_ab_t, in_=biasab)
            nc.sync.dma_start(out=bias_bl_t, in_=biasbl)
            nc.vector.memset(onesf, 1.0)
            nc.vector.memset(zerof, 0.0)
            nc.vector.tensor_copy(ones_r, onesf)
            # ones columns of v (offsets 64 and 256), zero gaps (65:128, 257:320)
            nc.vector.tensor_copy(v[:, :, 64:65], ones_r[:, 0:NJC])
            nc.vector.tensor_copy(v[:, :, 256:257], ones_r[:, 0:NJC])
            for jc in range(NJC):
                nc.vector.tensor_copy(v[:, jc, 65:128], zerof)
                nc.vector.tensor_copy(v[:, jc, 257:320], zerof)

            voff = (0, 128, 192, 320)
            with tc.tile_pool(name="dpool", bufs=4) as dpool, \
                 tc.tile_pool(name="spool", bufs=2) as spool, \
                 tc.tile_pool(name="epool", bufs=5) as epool, \
                 tc.tile_pool(name="rpool", bufs=2) as rpool, \
                 tc.tile_pool(name="gstage", bufs=2) as gstage, \
                 tc.tile_pool(name="ostage", bufs=2) as ostage, \
                 tc.tile_pool(name="ps_s", bufs=4, space="PSUM") as ps_s, \
                 tc.tile_pool(name="ps_o", bufs=2, space="PSUM") as ps_o:
                dm = {}
                opsum = {}

                def attn_tiles(ec, ic, jcs):
                    isl = slice(ic * 512, (ic + 1) * 512)
                    vb = 192 * ec
                    if (ec, ic) not in opsum:
                        oA = ps_o.tile([128, 512], F32, tag="po", name="oA")
                        oB = ps_o.tile([128, 512], F32, tag="po", name="oB")
                        opsum[(ec, ic)] = (oA, oB)
                    oA, oB = opsum[(ec, ic)]
                    for jc in jcs:
                        if jc // 4 == ic and jc not in dm:
                            dmt = dpool.tile([128, 512], F32, tag="dm", name="dm")
                            nc.sync.dma_start(out=dmt, in_=dmask[jc])
                            dm[jc] = dmt
                        jsl = slice(jc * 128, (jc + 1) * 128)
                        sA = ps_s.tile([128, 512], F32, tag="ps", name="sA")
                        nc.tensor.matmul(
                            sA, kt[ec][0:64, jsl], qt[ec][0:64, isl],
                            start=True, stop=True,
                        )
                        sB = ps_s.tile([128, 512], F32, tag="ps", name="sB")
                        nc.tensor.matmul(
                            sB, kt[ec][64:128, jsl], qt[ec][64:128, isl],
                            start=True, stop=True,
                        )
                        for S, vsl, odst in (
                            (sA, v[:, jc, vb : vb + 65], oA[0:65, :]),
                            (sB, v[:, jc, vb + 64 : vb + 192], oB),
                        ):
                            e = epool.tile([128, 512], F32R, tag="e", name="e")
                            if jc // 4 == ic:
                                st = spool.tile([128, 512], F32, tag="st", name="st")
                                nc.vector.tensor_add(st, S, dm[jc])
                                nc.scalar.activation(e, st, EXP)
                            else:
                                bt = bias_ab_t if jc // 4 > ic else bias_bl_t
                                nc.scalar.activation(
                                    e, S, EXP, bias=bt[:, jc : jc + 1], scale=1.0
                                )
                            nc.tensor.matmul(
                                odst, vsl, e, start=(jc == 0), stop=(jc == NJC - 1)
                            )

                def gate(ec, ic, ps_c):
                    isl = slice(ic * 512, (ic + 1) * 512)
                    oA, oB = opsum.pop((ec, ic))
                    rec = rpool.tile([128, 512], F32R, tag="rec", name="rec")
                    with nc.allow_low_precision(reason="f32r rounding for matmul"):
                        nc.vector.reciprocal(rec[64:65, :], oA[64:65, :])
                        nc.vector.reciprocal(rec[0:1, :], oB[0:1, :])
                    pAc = ps_c.tile([128, 512], F32, tag="pc", name="pAc")
                    nc.tensor.matmul(
                        pAc, ones_r[64:65, :], rec[64:65, :], start=True, stop=True
                    )
                    pBc = ps_c.tile([128, 512], F32, tag="pc", name="pBc")
                    nc.tensor.matmul(
                        pBc, ones_r[0:1, :], rec[0:1, :], start=True, stop=True
                    )
                    t1 = gstage.tile([128, 512], F32, tag="t1", name="t1")
                    nc.vector.tensor_mul(t1[0:64, :], oA[0:64, :], ut[ec][0:64, isl])
                    nc.vector.tensor_mul(t1[64:128, :], oB[64:128, :], ut[ec][64:128, isl])
                    with nc.allow_low_precision(reason="f32r rounding for matmul"):
                        nc.vector.tensor_mul(g[ec][0:64, isl], t1[0:64, :], pAc[0:64, :])
                        nc.vector.tensor_mul(
                            g[ec][64:128, isl], t1[64:128, :], pBc[64:128, :]
                        )

                def wo_ic(ic, wps):
                    for ii in range(4):
                        lc = 4 * ic + ii
                        for fc in range(2):
                            p = wps.tile([128, 512], F32, tag="wp", name="wp")
                            for ec2 in range(2):
                                nc.tensor.matmul(
                                    p,
                                    g[ec2][:, lc * 128 : (lc + 1) * 128],
                                    wo_r[ec2][:, fc * 512 : (fc + 1) * 512],
                                    start=(ec2 == 0),
                                    stop=(ec2 == 1),
                                )
                            o = ostage.tile([128, 512], F32, tag="os", name="os")
                            nc.vector.tensor_copy(o, p)
                            nc.sync.dma_start(
                                out=out[lc * 128 : (lc + 1) * 128, fc * 512 : (fc + 1) * 512],
                                in_=o,
                            )

                # ---- phase 1 (attention block (0,0) interleaved) ----
                with tc.tile_pool(name="xtw", bufs=1) as xtw, \
                     tc.tile_pool(name="xp", bufs=2) as xp, \
                     tc.tile_pool(name="land", bufs=4) as land, \
                     tc.tile_pool(name="wol", bufs=1) as wol, \
                     tc.tile_pool(name="pp", bufs=2, space="PSUM") as pp, \
                     tc.tile_pool(name="ppv", bufs=1, space="PSUM") as ppv:
                    w_r = {
                        nm: [xtw.tile([128, E], F32R, tag=f"w{nm}{dc}", name=f"w{nm}{dc}") for dc in range(NDC)]
                        for nm in ("k", "v", "q", "u")
                    }

                    def load_w(nm, dram):
                        for dc in range(NDC):
                            t = land.tile([128, E], F32, tag="land", name="wland")
                            nc.sync.dma_start(out=t, in_=dram[dc * 128 : (dc + 1) * 128, :])
                            nc.vector.tensor_copy(w_r[nm][dc], t)

                    def load_x(ic):
                        isl = slice(ic * 512, (ic + 1) * 512)
                        xtl = []
                        for dc in range(NDC):
                            t = land.tile([128, 512], F32, tag="land", name="xland")
                            nc.sync.dma_start(out=t, in_=xt[dc * 128 : (dc + 1) * 128, isl])
                            xr = xp.tile([128, 512], F32R, tag=f"xr{dc}", name=f"xr{dc}")
                            nc.vector.tensor_copy(xr, t)
                            xtl.append(xr)
                        return xtl

                    def p1(ic, xtl):
                        isl = slice(ic * 512, (ic + 1) * 512)
                        for nm, dest in (("k", kt), ("q", qt), ("u", ut)):
                            for ec in range(2):
                                p = pp.tile([128, 512], F32, tag="pp", name="pp")
                                for dc in range(NDC):
                                    nc.tensor.matmul(
                                        p,
                                        w_r[nm][dc][:, ec * 128 : (ec + 1) * 128],
                                        xtl[dc],
                                        start=(dc == 0),
                                        stop=(dc == NDC - 1),
                                    )
                                nc.vector.tensor_copy(dest[ec][:, isl], p)
                        for ii in range(4):
                            lc = 4 * ic + ii
                            p = pp.tile([128, E], F32, tag="pp", name="ppv")
                            for dc in range(NDC):
                                nc.tensor.matmul(
                                    p,
                                    xtl[dc][:, ii * 128 : (ii + 1) * 128],
                                    w_r["v"][dc],
                                    start=(dc == 0),
                                    stop=(dc == NDC - 1),
                                )
                            for hh in range(HPC):
                                nc.vector.tensor_copy(
                                    v[:, lc, voff[hh] : voff[hh] + 64],
                                    p[:, hh * 64 : (hh + 1) * 64],
                                )

                    load_w("k", wk)
                    xtl_cur = load_x(0)
                    load_w("q", wq)
                    load_w("v", wv)
                    load_w("u", wu)
                    for ec in range(2):
                        t = wol.tile([128, D], F32, tag="wol", name="woland")
                        nc.sync.dma_start(out=t, in_=wo[ec * 128 : (ec + 1) * 128, :])
                        nc.vector.tensor_copy(wo_r[ec], t)

                    xtl_next = load_x(1)
                    p1(0, xtl_cur)
                    xtl_cur, xtl_next = xtl_next, load_x(2)
                    p1(1, xtl_cur)
                    attn_tiles(0, 0, range(0, 8))
                    xtl_cur, xtl_next = xtl_next, load_x(3)
                    p1(2, xtl_cur)
                    attn_tiles(0, 0, range(8, 12))
                    p1(3, xtl_next)
                    attn_tiles(0, 0, range(12, 16))

                # ---- rest of attention + fused W_o ----
                with tc.tile_pool(name="ps_c", bufs=1, space="PSUM") as ps_c, \
                     tc.tile_pool(name="wps", bufs=1, space="PSUM") as wps:
                    gate(0, 0, ps_c)
                    attn_tiles(1, 0, range(NJC))
                    gate(1, 0, ps_c)
                    wo_ic(0, wps)
                    for ic in range(1, NIC):
                        dm.clear()
                        for ec in range(2):
                            attn_tiles(ec, ic, range(NJC))
                            gate(ec, ic, ps_c)
                        wo_ic(ic, wps)

    nc.compile()
    return nc


def _host_inputs(x, token_types, seq_lens, W_q, W_k, W_v, W_u, W_o):
    x = np.asarray(x, dtype=np.float32)
    token_types = np.asarray(token_types)
    seq_lens = np.asarray(seq_lens)
    W_q = np.asarray(W_q, dtype=np.float32)
    W_k = np.asarray(W_k, dtype=np.float32)
    W_v = np.asarray(W_v, dtype=np.float32)
    W_u = np.asarray(W_u, dtype=np.float32)
    W_o = np.asarray(W_o, dtype=np.float32)

    per_batch = []
    jr = np.arange(L)
    for b in range(B):
        xt = np.ascontiguousarray(x[b].T)
        prompt = token_types[b] < 3
        valid = jr < int(seq_lens[b])
        ab = np.where(prompt & valid, 0.0, NEG).astype(np.float32)
        bl = np.where(valid, 0.0, NEG).astype(np.float32)
        biasab = np.ascontiguousarray(ab.reshape(NJC, 128).T)
        biasbl = np.ascontiguousarray(bl.reshape(NJC, 128).T)
        dmk = np.empty((NJC, 128, 512), np.float32)
        for jc in range(NJC):
            j = jr[jc * 128 : (jc + 1) * 128]
            i = np.arange((jc // 4) * 512, (jc // 4) * 512 + 512)
            allowed = valid[j][:, None] & (prompt[j][:, None] | (j[:, None] <= i[None, :]))
            dmk[jc] = np.where(allowed, 0.0, NEG)
        per_batch.append((xt, biasab, biasbl, dmk))

    in_maps = []
    for c in range(8):
        b, gi = c // 4, c % 4
        e0 = E * gi
        xt, biasab, biasbl, dmk = per_batch[b]
        in_maps.append(
            {
                "xt": xt,
                "wq": np.ascontiguousarray((W_q[e0 : e0 + E] / 8.0).T),
                "wk": np.ascontiguousarray(W_k[e0 : e0 + E].T),
                "wv": np.ascontiguousarray(W_v[e0 : e0 + E].T),
                "wu": np.ascontiguousarray(W_u[e0 : e0 + E].T),
                "wo": np.ascontiguousarray(W_o[:, e0 : e0 + E].T),
                "biasab": biasab,
                "biasbl": biasbl,
                "dmask": dmk,
            }
        )
    return in_maps


def kernel(x, token_types, seq_lens, W_q, W_k, W_v, W_u, W_o, **_run_kwargs):
    if "nc" not in _cache:
        _cache["nc"] = _build()
    nc = _cache["nc"]
    in_maps = _host_inputs(x, token_types, seq_lens, W_q, W_k, W_v, W_u, W_o)
    try:
        res = run_bass_kernel_spmd(nc, in_maps, list(range(8)), **_run_kwargs)
    except Exception as ex:  # transient NRT device wedge: retry once
        if "UNRECOVERABLE" not in str(ex) and "UNAVAILABLE" not in str(ex):
            raise
        res = run_bass_kernel_spmd(nc, in_maps, list(range(8)), **_run_kwargs)
    _cache["last_result"] = res
    full = np.zeros((B, L, D), np.float64)
    for c in range(8):
        full[c // 4] += res.results[c]["out"].astype(np.float64)
    return full.astype(np.float32)



# revision 10
# speedup vs baseline: 1.3271x; 1.3271x over previous
"""HSTU attention (B=2, L=2048, D=1024, H=16) on 8 TRN2 NeuronCores.

Sharding: batch (2) x head-group (4 heads, 256 features) -> 8 cores.

Per core, for its batch b and 4 heads:
  - Projections run as 3-term fp8 DoubleRow matmuls: x and 16*W are sent as
    fp8 (hi) plus fp8 residual (lo); psum accumulates hi*hi + hi*lo + lo*hi
    (the dropped lo*lo term is ~1e-3 relative).  0.75x the cycles of bf16
    at bf16-class accuracy; the 1/16 is folded into the psum->SBUF copies.
  - Scores S^T = K^T.T @ Q in bf16, [keys x queries] layout, psum tiles of
    [128, 1024] (2 banks); exp(S/8) on ACT (scale=0.125) -> bf16 e tiles.
  - Key chunks beyond max(seq_len) are skipped (runtime-specialized NJ);
    masking is folded into the AV operands: V is premasked into vF (valid)
    and vP (prompt&valid), true-diagonal 128x128 blocks get a {0,1} mask
    multiply (Pool engine), row sums use mask columns.
  - AV is swapped: out[tokens, feats] += e_chunk.T @ v (N=64), with N=1
    row-sum matmuls into a shared psum bank; softmax normalization + U
    gating is a per-partition scalar_tensor_tensor from an SBUF copy.
  - g is transposed per 128x128 chunk: DMA xbar transpose for the first
    half (ec0, mid-kernel), PE transpose via identity for the tail half.
  - W_o partials per ec-half in bf16; outputs land in two bf16 partial
    tensors, DMA'd four token-chunks at a time.
Host sums the 8 partial outputs per batch.

Scheduling: a software-pipelined (jc, query-half) unit loop per head with
hooks spreading projections / W_o groups into PE slack; per-chunk SBUF
tiles avoid false tile-granularity dependencies; a warm-up matmul chain
brings the PE out of its low p-state during the initial DMA window.
"""

import sys

for _p in ("/opt/trn_rl_repo", "/root/.axon_site/_ro/trn_rl_repo"):
    if _p not in sys.path:
        sys.path.insert(0, _p)

import numpy as np
import ml_dtypes

import concourse.bass as bass  # noqa: F401
import concourse.mybir as mybir
import concourse.tile as tile
from concourse import bacc
from concourse.bass_utils import run_bass_kernel_spmd

F32 = mybir.dt.float32
BF16 = mybir.dt.bfloat16
F8 = mybir.dt.float8e4
EXP = mybir.ActivationFunctionType.Exp
COPY = mybir.ActivationFunctionType.Copy
DR = mybir.MatmulPerfMode.DoubleRow
MULT = mybir.AluOpType.mult

B, L, D, H = 2, 2048, 1024, 16
DK = D // H          # 64
HPC = 4              # heads per core
E = HPC * DK         # 256 features per core
NDC = D // 128       # 8 contraction chunks for projections
NLC = L // 128       # 16 token chunks
NIC = L // 512       # 4 token 512-spans

_cache = {}


def _build(NJ):
    NLK = NJ * 128
    kspans = [(s, min(512, NLK - s)) for s in range(0, NLK, 512)]

    nc = bacc.Bacc("TRN2", target_bir_lowering=False, debug=False)

    xd = {
        t: nc.dram_tensor(f"x_{t}", [128, NDC, L], F8, kind="ExternalInput").ap()
        for t in ("h", "l")
    }
    wd = {
        (nm, t): nc.dram_tensor(f"w{nm}_{t}", [128, NDC, E], F8, kind="ExternalInput").ap()
        for nm in ("q", "k", "v", "u") for t in ("h", "l")
    }
    wo16d = nc.dram_tensor("wo16", [128, 2, D], BF16, kind="ExternalInput").ap()
    dm16d = nc.dram_tensor("dm16", [128, NJ, 128], BF16, kind="ExternalInput").ap()
    mc16d = nc.dram_tensor("mc16", [128, NJ, 3], BF16, kind="ExternalInput").ap()
    mxfd = nc.dram_tensor("mxf", [128, NJ, 2], F32, kind="ExternalInput").ap()
    identd = nc.dram_tensor("ident", [128, 128], BF16, kind="ExternalInput").ap()
    outd = [
        nc.dram_tensor(f"out{ec}", [L, D], BF16, kind="ExternalOutput").ap()
        for ec in range(2)
    ]
    # out viewed as [tok-in-chunk 128, chunk 16, feat 1024] for merged DMAs
    outr = [o.rearrange("(a p) d -> p a d", p=128) for o in outd]

    with tile.TileContext(nc) as tc:
        with tc.tile_pool(name="persist", bufs=1) as persist, \
             tc.tile_pool(name="e8p", bufs=4) as e8p, \
             tc.tile_pool(name="eDp", bufs=2) as eDp, \
             tc.tile_pool(name="osb", bufs=2) as osb:
            xs = {
                (s, t): persist.tile([128, NDC, 512], F8, tag=f"xs{s}{t}", name=f"xs{s}{t}")
                for s in range(NIC) for t in ("h", "l")
            }
            w8 = {
                k: persist.tile([128, NDC, E], F8, tag=f"w{k[0]}{k[1]}", name=f"w{k[0]}{k[1]}")
                for k in wd
            }
            wo16 = persist.tile([128, 2, D], BF16, tag="wo16", name="wo16")
            dm16 = persist.tile([128, NJ, 128], BF16, tag="dm16", name="dm16")
            mc16 = persist.tile([128, NJ, 3], BF16, tag="mc16", name="mc16")
            mxf = persist.tile([128, NJ, 2], F32, tag="mxf", name="mxf")
            ident = persist.tile([128, 128], BF16, tag="ident", name="ident")
            wtmp = persist.tile([128, 512], BF16, tag="wtmp", name="wtmp")
            q16 = [persist.tile([128, L], BF16, tag=f"q16_{ec}", name=f"q16_{ec}")
                   for ec in range(2)]
            k16 = [persist.tile([128, NLK], BF16, tag=f"k16_{ec}", name=f"k16_{ec}")
                   for ec in range(2)]
            u16 = [persist.tile([128, E], BF16, tag=f"u16_{lc}", name=f"u16_{lc}")
                   for lc in range(NLC)]
            vF8 = [persist.tile([128, E], BF16, tag=f"vF_{jc}", name=f"vF_{jc}")
                   for jc in range(NJ)]
            vP8 = [persist.tile([128, E], BF16, tag=f"vP_{jc}", name=f"vP_{jc}")
                   for jc in range(NJ)]
            g16 = [persist.tile([128, E], BF16, tag=f"g_{lc}", name=f"g_{lc}")
                   for lc in range(NLC)]
            gT16 = {(ec, lc): persist.tile([128, 128], BF16, tag=f"gt{ec}_{lc}", name=f"gt{ec}_{lc}")
                    for ec in range(2) for lc in range(NLC)}
            avs = persist.tile([128, 1024], F32, tag="avs", name="avs")
            rec16 = [persist.tile([128, 16], F32, tag=f"rec{p}", name=f"rec{p}")
                     for p in range(2)]

            # -------- emission helpers --------
            def dma_x(si):
                s0 = si * 512
                for t in ("h", "l"):
                    nc.sync.dma_start(out=xs[(si, t)], in_=xd[t][:, :, s0 : s0 + 512])

            def proj_mms(p, w, lhs_of, rhs_of):
                """3-term hi/lo DR accumulation into psum slice p[:, 0:w]."""
                terms = (("h", "h"), ("h", "l"), ("l", "h"))
                n = NDC // 2
                first = True
                for (tx, tw) in terms:
                    for t in range(n):
                        nc.tensor.matmul(
                            p[:, 0:w],
                            lhs_of(tx, tw, t),
                            rhs_of(tx, tw, t),
                            start=first,
                            stop=(tx, tw) == ("l", "h") and t == n - 1,
                            perf_mode=DR,
                        )
                        first = False

            def proj_qk(pool, nm, ec, c0, w):
                """q16/k16[ec][:, c0:c0+w] = (x @ (16W).T)/16 in [feat, tok]."""
                p = pool.tile([128, 512], F32, tag="pp", name="pp")
                si, o = c0 // 512, c0 % 512
                proj_mms(
                    p, w,
                    lambda tx, tw, t: w8[(nm, tw)][:, 2 * t : 2 * t + 2, ec * 128 : (ec + 1) * 128],
                    lambda tx, tw, t: xs[(si, tx)][:, 2 * t : 2 * t + 2, o : o + w],
                )
                dest = q16 if nm == "q" else k16
                with nc.allow_low_precision(reason="bf16 store"):
                    nc.vector.tensor_scalar_mul(
                        dest[ec][:, c0 : c0 + w], p[:, 0:w], 1.0 / 16.0
                    )

            def proj_v(pool, h, jc):
                hsl = slice(64 * h, 64 * h + 64)
                si, o = (jc * 128) // 512, (jc * 128) % 512
                p = pool.tile([128, 512], F32, tag="pp", name="pp")
                proj_mms(
                    p, 64,
                    lambda tx, tw, t: xs[(si, tx)][:, 2 * t : 2 * t + 2, o : o + 128],
                    lambda tx, tw, t: w8[("v", tw)][:, 2 * t : 2 * t + 2, hsl],
                )
                with nc.allow_low_precision(reason="bf16 store"):
                    nc.vector.tensor_scalar_mul(
                        vF8[jc][:, hsl], p[:, 0:64], mxf[:, jc, 0:1]
                    )
                    nc.vector.tensor_scalar_mul(
                        vP8[jc][:, hsl], p[:, 0:64], mxf[:, jc, 1:2]
                    )

            def proj_u(pool, h, lc):
                hsl = slice(64 * h, 64 * h + 64)
                si, o = (lc * 128) // 512, (lc * 128) % 512
                p = pool.tile([128, 512], F32, tag="pp", name="pp")
                proj_mms(
                    p, 64,
                    lambda tx, tw, t: xs[(si, tx)][:, 2 * t : 2 * t + 2, o : o + 128],
                    lambda tx, tw, t: w8[("u", tw)][:, 2 * t : 2 * t + 2, hsl],
                )
                with nc.allow_low_precision(reason="bf16 store"):
                    nc.vector.tensor_scalar_mul(
                        u16[lc][:, hsl], p[:, 0:64], 1.0 / 16.0
                    )

            def scores_exp(scp, h, jc, half):
                """e tile [128 keys, 1024 queries] = exp(S/8) for (h, jc, half)."""
                ec, hh = h // 2, h % 2
                jsl = slice(jc * 128, (jc + 1) * 128)
                e8 = e8p.tile([128, 1024], BF16, tag="e8", name="e8")
                sc = scp.tile([128, 1024], F32, tag="sc", name="sc")
                for q in range(2):
                    q0 = half * 1024 + q * 512
                    nc.tensor.matmul(
                        sc[:, q * 512 : (q + 1) * 512],
                        k16[ec][64 * hh : 64 * hh + 64, jsl],
                        q16[ec][64 * hh : 64 * hh + 64, q0 : q0 + 512],
                        start=True, stop=True,
                    )
                with nc.allow_low_precision(reason="bf16 exp"):
                    nc.scalar.activation(e8, sc, EXP, scale=0.125)
                return e8

            def av_half(av, rs, h, jc, half, e8):
                hsl = slice(64 * h, 64 * h + 64)
                base = half * 8
                eD = None
                if base <= jc < base + 8:
                    eD = eDp.tile([128, 128], BF16, tag="eD", name="eD")
                    loc = jc * 128 - half * 1024
                    with nc.allow_low_precision(reason="mask mul"):
                        nc.gpsimd.tensor_mul(
                            eD, e8[:, loc : loc + 128], dm16[:, jc, :]
                        )
                for lc in range(base, base + 8):
                    loc = lc * 128 - half * 1024
                    if jc == lc:
                        lhsT, vt, mcol = eD, vF8, 2
                    elif jc < lc:
                        lhsT, vt, mcol = e8[:, loc : loc + 128], vF8, 0
                    else:
                        lhsT, vt, mcol = e8[:, loc : loc + 128], vP8, 1
                    nc.tensor.matmul(
                        av[:, lc * 64 : (lc + 1) * 64],
                        lhsT, vt[jc][:, hsl],
                        start=(jc == 0 and lc == base),
                        stop=(jc == NJ - 1 and lc == base + 7),
                    )
                    nc.tensor.matmul(
                        rs[:, (h % 2) * 16 + lc : (h % 2) * 16 + lc + 1],
                        lhsT, mc16[:, jc, mcol : mcol + 1],
                        start=(jc == 0 and half == 0 and lc == 0),
                        stop=(jc == NJ - 1 and lc == NLC - 1),
                    )

            def head_att(scp, projp, av, rs, h, pre=(), hooks=None):
                hooks = hooks or {}
                pend = []
                ui = 0
                for half in range(2):
                    for jc in range(NJ):
                        e = scores_exp(scp, h, jc, half)
                        if half == 0:
                            proj_v(projp, h, jc)
                        if ui == 0:
                            for f in pre:
                                f()
                        if len(pend) >= 2:
                            av_half(av, rs, h, *pend.pop(0))
                        for f in hooks.get(ui, ()):
                            f()
                        pend.append((jc, half, e))
                        ui += 1
                for item in pend:
                    av_half(av, rs, h, *item)

            def gate(av, rs, h):
                p = h % 2
                with nc.allow_low_precision(reason="gate"):
                    nc.vector.reciprocal(rec16[p], rs[:, p * 16 : (p + 1) * 16])
                    nc.vector.tensor_copy(avs, av)
                    for lc in range(NLC):
                        nc.vector.scalar_tensor_tensor(
                            g16[lc][:, 64 * h : 64 * h + 64],
                            avs[:, lc * 64 : (lc + 1) * 64],
                            rec16[p][:, lc : lc + 1],
                            u16[lc][:, 64 * h : 64 * h + 64],
                            MULT, MULT,
                        )

            def transposes_dma(ec):
                for lc in range(NLC):
                    nc.sync.dma_start_transpose(
                        gT16[(ec, lc)],
                        g16[lc][:, ec * 128 : (ec + 1) * 128],
                    )

            wo_alt = [0]

            def wo_pair(wop, o, ec, lc, slot, tail):
                """two W_o matmuls for one token chunk into o[:, slot, :]."""
                for fc in range(2):
                    p = wop.tile([128, 512], F32, tag="pp", name="pp")
                    nc.tensor.matmul(
                        p,
                        gT16[(ec, lc)],
                        wo16[:, ec, fc * 512 : (fc + 1) * 512],
                        start=True, stop=True,
                    )
                    wo_alt[0] += 1
                    with nc.allow_low_precision(reason="bf16 out"):
                        if tail and wo_alt[0] % 2 == 0:
                            nc.scalar.activation(
                                o[:, slot, fc * 512 : (fc + 1) * 512], p, COPY
                            )
                        else:
                            nc.vector.tensor_copy(
                                o[:, slot, fc * 512 : (fc + 1) * 512], p
                            )

            def wo_quad(wop, ec, q, tail=False):
                """4 token chunks -> one merged out DMA."""
                o = osb.tile([128, 4, 1024], BF16, tag="osb", name="osb")
                for s in range(4):
                    wo_pair(wop, o, ec, 4 * q + s, s, tail)
                nc.sync.dma_start(
                    out=outr[ec][:, 4 * q : 4 * q + 4, :], in_=o
                )

            NU = 2 * NJ  # units per head

            def spread(jobs, lo, hi):
                hooks = {}
                n = len(jobs)
                for i, job in enumerate(jobs):
                    hooks.setdefault(lo + (i * (hi - lo)) // n, []).append(job)
                return hooks

            with tc.tile_pool(name="av", bufs=1, space="PSUM") as avp, \
                 tc.tile_pool(name="rs", bufs=1, space="PSUM") as rsp:
                av = avp.tile([128, 1024], F32, tag="av", name="av")
                rs = rsp.tile([128, 32], F32, tag="rs", name="rs")

                # -------- phase 1: warmup, DMAs, h0, QK proj, U(h0) --------
                with tc.tile_pool(name="pp", bufs=3, space="PSUM") as pp, \
                     tc.tile_pool(name="sc1", bufs=1, space="PSUM") as sc1:
                    # PE warm-up chain during the initial DMA window
                    nc.vector.memset(wtmp, 0.0)
                    wp = pp.tile([128, 512], F32, tag="pp", name="pp")
                    for i in range(5):
                        nc.tensor.matmul(
                            wp, wtmp[:, 0:128], wtmp,
                            start=(i == 0), stop=(i == 4),
                        )

                    # input DMAs (x on SP queue, weights/masks on ACT queue)
                    dma_x(0)
                    for t in ("h", "l"):
                        nc.scalar.dma_start(out=w8[("k", t)], in_=wd[("k", t)])
                    for t in ("h", "l"):
                        nc.scalar.dma_start(out=w8[("q", t)], in_=wd[("q", t)])
                    dma_x(1)
                    for t in ("h", "l"):
                        nc.scalar.dma_start(out=w8[("v", t)], in_=wd[("v", t)])
                    for t in ("h", "l"):
                        nc.scalar.dma_start(out=w8[("u", t)], in_=wd[("u", t)])
                    nc.scalar.dma_start(out=dm16, in_=dm16d)
                    nc.scalar.dma_start(out=mc16, in_=mc16d)
                    nc.scalar.dma_start(out=mxf, in_=mxfd)
                    nc.scalar.dma_start(out=wo16, in_=wo16d)
                    nc.scalar.dma_start(out=ident, in_=identd)

                    proj_qk(pp, "k", 0, 0, 512)
                    proj_qk(pp, "q", 0, 0, 512)
                    proj_qk(pp, "q", 0, 512, 512)

                    jobs0 = []
                    jobs0.append(lambda: dma_x(2))
                    for (c0, w) in kspans[1:2]:
                        jobs0.append(lambda c0=c0, w=w: proj_qk(pp, "k", 0, c0, w))
                    jobs0.append(lambda: proj_qk(pp, "q", 0, 1024, 512))
                    jobs0.append(lambda: dma_x(3))
                    for (c0, w) in kspans[2:]:
                        jobs0.append(lambda c0=c0, w=w: proj_qk(pp, "k", 0, c0, w))
                    jobs0.append(lambda: proj_qk(pp, "q", 0, 1536, 512))
                    for lc in range(NLC):
                        jobs0.append(lambda lc=lc: proj_u(pp, 0, lc))
                    for (c0, w) in kspans:
                        jobs0.append(lambda c0=c0, w=w: proj_qk(pp, "k", 1, c0, w))
                    for ic in range(NIC):
                        jobs0.append(lambda ic=ic: proj_qk(pp, "q", 1, ic * 512, 512))
                    head_att(sc1, pp, av, rs, 0, hooks=spread(jobs0, 1, NU))

                # -------- phase 2: h1-h3, ec0 wo --------
                with tc.tile_pool(name="sc2", bufs=2, space="PSUM") as sc2, \
                     tc.tile_pool(name="wop", bufs=1, space="PSUM") as wop:
                    jobs1 = [lambda lc=lc: proj_u(wop, 1, lc) for lc in range(NLC)]
                    head_att(sc2, wop, av, rs, 1,
                             pre=[lambda: gate(av, rs, 0)],
                             hooks=spread(jobs1, 1, NU))

                    jobs2 = [lambda lc=lc: proj_u(wop, 2, lc) for lc in range(NLC)]
                    jobs2 += [lambda q=q: wo_quad(wop, 0, q) for q in range(2)]
                    head_att(sc2, wop, av, rs, 2,
                             pre=[lambda: gate(av, rs, 1), lambda: transposes_dma(0)],
                             hooks=spread(jobs2, 1, NU))

                    jobs3 = [lambda lc=lc: proj_u(wop, 3, lc) for lc in range(NLC)]
                    jobs3 += [lambda q=q: wo_quad(wop, 0, q) for q in range(2, 4)]
                    head_att(sc2, wop, av, rs, 3,
                             pre=[lambda: gate(av, rs, 2)],
                             hooks=spread(jobs3, 1, NU))
                    gate(av, rs, 3)

            # -------- phase 3: tail (av/rs closed): PE transposes + ec1 wo --------
            with tc.tile_pool(name="wo2", bufs=4, space="PSUM") as wo2, \
                 tc.tile_pool(name="tp", bufs=4, space="PSUM") as tpp:
                for lc in range(NLC):
                    t = tpp.tile([128, 128], BF16, tag="tp", name="tp")
                    nc.tensor.transpose(t, g16[lc][:, 128:256], ident)
                    with nc.allow_low_precision(reason="bf16 transpose"):
                        if lc % 2 == 0:
                            nc.vector.tensor_copy(gT16[(1, lc)], t)
                        else:
                            nc.scalar.activation(gT16[(1, lc)], t, COPY)
                for q in range(4):
                    wo_quad(wo2, 1, q, tail=True)

    nc.compile()
    return nc


def _hilo(a):
    f8 = ml_dtypes.float8_e4m3
    hi = a.astype(f8)
    lo = (a - hi.astype(np.float32)).astype(f8)
    return hi, lo


def _host_inputs(NJ, x, token_types, seq_lens, W_q, W_k, W_v, W_u, W_o):
    x = np.asarray(x, dtype=np.float32)
    token_types = np.asarray(token_types)
    seq_lens = np.asarray(seq_lens)
    W = {
        "q": np.asarray(W_q, dtype=np.float32),
        "k": np.asarray(W_k, dtype=np.float32),
        "v": np.asarray(W_v, dtype=np.float32),
        "u": np.asarray(W_u, dtype=np.float32),
    }
    W_o = np.asarray(W_o, dtype=np.float32)
    bf = ml_dtypes.bfloat16

    per_batch = []
    for b in range(B):
        xt = np.ascontiguousarray(x[b].T.reshape(NDC, 128, L).transpose(1, 0, 2))
        xh, xl = _hilo(xt)
        prompt = np.asarray(token_types[b] < 3)
        valid = np.arange(L) < int(seq_lens[b])
        dm16 = np.zeros((128, NJ, 128), bf)
        mc16 = np.zeros((128, NJ, 3), bf)
        mxf = np.zeros((128, NJ, 2), np.float32)
        for jc in range(NJ):
            j = np.arange(jc * 128, (jc + 1) * 128)
            i = j  # true-diagonal block
            allow = valid[j][:, None] & (prompt[j][:, None] | (j[:, None] <= i[None, :]))
            dm16[:, jc, :] = allow.astype(np.float32)
            mF = valid[j].astype(np.float32)
            mP = (valid[j] & prompt[j]).astype(np.float32)
            mc16[:, jc, 0] = mF
            mc16[:, jc, 1] = mP
            mc16[:, jc, 2] = 1.0
            mxf[:, jc, 0] = mF / 16.0
            mxf[:, jc, 1] = mP / 16.0
        per_batch.append((xh, xl, dm16, mc16, mxf))

    ident = np.eye(128, dtype=bf)
    in_maps = []
    for c in range(8):
        b, gi = c // 4, c % 4
        e0 = E * gi
        xh, xl, dm16, mc16, mxf = per_batch[b]
        m = {"x_h": xh, "x_l": xl, "dm16": dm16, "mc16": mc16, "mxf": mxf,
             "ident": ident}
        for nm in ("q", "k", "v", "u"):
            wt = np.ascontiguousarray(
                (W[nm][e0 : e0 + E] * 16.0).T.reshape(NDC, 128, E).transpose(1, 0, 2)
            )
            wh, wl = _hilo(wt)
            m[f"w{nm}_h"] = wh
            m[f"w{nm}_l"] = wl
        m["wo16"] = np.ascontiguousarray(
            W_o[:, e0 : e0 + E].T.reshape(2, 128, D).transpose(1, 0, 2)
        ).astype(bf)
        in_maps.append(m)
    return in_maps


def kernel(x, token_types, seq_lens, W_q, W_k, W_v, W_u, W_o, **_run_kwargs):
    seq = np.asarray(seq_lens)
    NJ = int(np.ceil(seq.max() / 128.0))
    NJ = max(1, min(NLC, NJ))
    if ("nc", NJ) not in _cache:
        _cache[("nc", NJ)] = _build(NJ)
    nc = _cache[("nc", NJ)]
    in_maps = _host_inputs(NJ, x, token_types, seq_lens, W_q, W_k, W_v, W_u, W_o)
    try:
        res = run_bass_kernel_spmd(nc, in_maps, list(range(8)), **_run_kwargs)
    except Exception as ex:  # transient NRT device wedge: retry once
        if "UNRECOVERABLE" not in str(ex) and "UNAVAILABLE" not in str(ex):
            raise
        res = run_bass_kernel_spmd(nc, in_maps, list(range(8)), **_run_kwargs)
    _cache["last_result"] = res
    _cache["nc"] = nc  # for test.py TimelineSim
    full = np.zeros((B, L, D), np.float64)
    for c in range(8):
        r = res.results[c]
        full[c // 4] += r["out0"].astype(np.float64) + r["out1"].astype(np.float64)
    return full.astype(np.float32)


# revision 11
# speedup vs baseline: 1.3297x; 1.0019x over previous
"""HSTU attention (B=2, L=2048, D=1024, H=16) on 8 TRN2 NeuronCores.

Sharding: batch (2) x head-group (4 heads, 256 features) -> 8 cores.

Per core, for its batch b and 4 heads:
  - Projections run as 3-term fp8 DoubleRow matmuls: x and 16*W are sent as
    fp8 (hi) plus fp8 residual (lo); psum accumulates hi*hi + hi*lo + lo*hi
    (the dropped lo*lo term is ~1e-3 relative).  0.75x the cycles of bf16
    at bf16-class accuracy; the 1/16 is folded into the psum->SBUF copies.
  - Scores S^T = K^T.T @ Q in bf16, [keys x queries] layout, psum tiles of
    [128, 1024] (2 banks); exp(S/8) on ACT (scale=0.125) -> bf16 e tiles.
  - Key chunks beyond max(seq_len) are skipped (runtime-specialized NJ);
    masking is folded into the AV operands: V is premasked into vF (valid)
    and vP (prompt&valid), true-diagonal 128x128 blocks get a {0,1} mask
    multiply (Pool engine), row sums use mask columns.
  - AV is swapped: out[tokens, feats] += e_chunk.T @ v (N=64), with N=1
    row-sum matmuls into a shared psum bank; softmax normalization + U
    gating is a per-partition scalar_tensor_tensor from an SBUF copy.
  - g is transposed per 128x128 chunk: DMA xbar transpose for the first
    half (ec0, mid-kernel), PE transpose via identity for the tail half.
  - W_o partials per ec-half in bf16; outputs land in two bf16 partial
    tensors, DMA'd four token-chunks at a time.
Host sums the 8 partial outputs per batch.

Scheduling: a software-pipelined (jc, query-half) unit loop per head with
hooks spreading projections / W_o groups into PE slack; per-chunk SBUF
tiles avoid false tile-granularity dependencies; a warm-up matmul chain
brings the PE out of its low p-state during the initial DMA window.
"""

import sys

for _p in ("/opt/trn_rl_repo", "/root/.axon_site/_ro/trn_rl_repo"):
    if _p not in sys.path:
        sys.path.insert(0, _p)

import numpy as np
import ml_dtypes

import concourse.bass as bass  # noqa: F401
import concourse.mybir as mybir
import concourse.tile as tile
from concourse import bacc
from concourse.bass_utils import run_bass_kernel_spmd

F32 = mybir.dt.float32
BF16 = mybir.dt.bfloat16
F8 = mybir.dt.float8e4
EXP = mybir.ActivationFunctionType.Exp
COPY = mybir.ActivationFunctionType.Copy
DR = mybir.MatmulPerfMode.DoubleRow
MULT = mybir.AluOpType.mult

B, L, D, H = 2, 2048, 1024, 16
DK = D // H          # 64
HPC = 4              # heads per core
E = HPC * DK         # 256 features per core
NDC = D // 128       # 8 contraction chunks for projections
NLC = L // 128       # 16 token chunks
NIC = L // 512       # 4 token 512-spans

_cache = {}


def _build(NJ):
    NLK = NJ * 128
    kspans = [(s, min(512, NLK - s)) for s in range(0, NLK, 512)]

    nc = bacc.Bacc("TRN2", target_bir_lowering=False, debug=False)

    xd = {
        t: nc.dram_tensor(f"x_{t}", [128, NDC, L], F8, kind="ExternalInput").ap()
        for t in ("h", "l")
    }
    wd = {
        (nm, t): nc.dram_tensor(f"w{nm}_{t}", [128, NDC, E], F8, kind="ExternalInput").ap()
        for nm in ("q", "k", "v", "u") for t in ("h", "l")
    }
    wo16d = nc.dram_tensor("wo16", [128, 2, D], BF16, kind="ExternalInput").ap()
    dm16d = nc.dram_tensor("dm16", [128, NJ, 128], BF16, kind="ExternalInput").ap()
    mc16d = nc.dram_tensor("mc16", [128, NJ, 3], BF16, kind="ExternalInput").ap()
    mxfd = nc.dram_tensor("mxf", [128, NJ, 2], F32, kind="ExternalInput").ap()
    identd = nc.dram_tensor("ident", [128, 128], BF16, kind="ExternalInput").ap()
    outd = [
        nc.dram_tensor(f"out{ec}", [L, D], BF16, kind="ExternalOutput").ap()
        for ec in range(2)
    ]
    # out viewed as [tok-in-chunk 128, chunk 16, feat 1024] for merged DMAs
    outr = [o.rearrange("(a p) d -> p a d", p=128) for o in outd]

    with tile.TileContext(nc) as tc:
        with tc.tile_pool(name="persist", bufs=1) as persist, \
             tc.tile_pool(name="e8p", bufs=4) as e8p, \
             tc.tile_pool(name="eDp", bufs=2) as eDp, \
             tc.tile_pool(name="osb", bufs=2) as osb:
            xs = {
                (s, t): persist.tile([128, NDC, 512], F8, tag=f"xs{s}{t}", name=f"xs{s}{t}")
                for s in range(NIC) for t in ("h", "l")
            }
            w8 = {
                k: persist.tile([128, NDC, E], F8, tag=f"w{k[0]}{k[1]}", name=f"w{k[0]}{k[1]}")
                for k in wd
            }
            wo16 = persist.tile([128, 2, D], BF16, tag="wo16", name="wo16")
            dm16 = persist.tile([128, NJ, 128], BF16, tag="dm16", name="dm16")
            mc16 = persist.tile([128, NJ, 3], BF16, tag="mc16", name="mc16")
            mxf = persist.tile([128, NJ, 2], F32, tag="mxf", name="mxf")
            ident = persist.tile([128, 128], BF16, tag="ident", name="ident")
            wtmp = persist.tile([128, 512], BF16, tag="wtmp", name="wtmp")
            q16 = [persist.tile([128, L], BF16, tag=f"q16_{ec}", name=f"q16_{ec}")
                   for ec in range(2)]
            k16 = [persist.tile([128, NLK], BF16, tag=f"k16_{ec}", name=f"k16_{ec}")
                   for ec in range(2)]
            u16 = [persist.tile([128, E], BF16, tag=f"u16_{lc}", name=f"u16_{lc}")
                   for lc in range(NLC)]
            vF8 = [persist.tile([128, E], BF16, tag=f"vF_{jc}", name=f"vF_{jc}")
                   for jc in range(NJ)]
            vP8 = [persist.tile([128, E], BF16, tag=f"vP_{jc}", name=f"vP_{jc}")
                   for jc in range(NJ)]
            g16 = [persist.tile([128, E], BF16, tag=f"g_{lc}", name=f"g_{lc}")
                   for lc in range(NLC)]
            gT16 = {(ec, lc): persist.tile([128, 128], BF16, tag=f"gt{ec}_{lc}", name=f"gt{ec}_{lc}")
                    for ec in range(2) for lc in range(NLC)}
            avs = persist.tile([128, 1024], F32, tag="avs", name="avs")
            rec16 = [persist.tile([128, 16], F32, tag=f"rec{p}", name=f"rec{p}")
                     for p in range(2)]

            # -------- emission helpers --------
            def dma_x(si, which=("h", "l")):
                s0 = si * 512
                for t in which:
                    nc.sync.dma_start(out=xs[(si, t)], in_=xd[t][:, :, s0 : s0 + 512])

            def proj_mms(p, w, lhs_of, rhs_of):
                """3-term hi/lo DR accumulation into psum slice p[:, 0:w]."""
                terms = (("h", "h"), ("h", "l"), ("l", "h"))
                n = NDC // 2
                first = True
                for (tx, tw) in terms:
                    for t in range(n):
                        nc.tensor.matmul(
                            p[:, 0:w],
                            lhs_of(tx, tw, t),
                            rhs_of(tx, tw, t),
                            start=first,
                            stop=(tx, tw) == ("l", "h") and t == n - 1,
                            perf_mode=DR,
                        )
                        first = False

            def proj_qk(pool, nm, ec, c0, w):
                """q16/k16[ec][:, c0:c0+w] = (x @ (16W).T)/16 in [feat, tok]."""
                p = pool.tile([128, 512], F32, tag="pp", name="pp")
                si, o = c0 // 512, c0 % 512
                proj_mms(
                    p, w,
                    lambda tx, tw, t: w8[(nm, tw)][:, 2 * t : 2 * t + 2, ec * 128 : (ec + 1) * 128],
                    lambda tx, tw, t: xs[(si, tx)][:, 2 * t : 2 * t + 2, o : o + w],
                )
                dest = q16 if nm == "q" else k16
                with nc.allow_low_precision(reason="bf16 store"):
                    nc.vector.tensor_scalar_mul(
                        dest[ec][:, c0 : c0 + w], p[:, 0:w], 1.0 / 16.0
                    )

            def proj_v(pool, h, jc):
                hsl = slice(64 * h, 64 * h + 64)
                si, o = (jc * 128) // 512, (jc * 128) % 512
                p = pool.tile([128, 512], F32, tag="pp", name="pp")
                proj_mms(
                    p, 64,
                    lambda tx, tw, t: xs[(si, tx)][:, 2 * t : 2 * t + 2, o : o + 128],
                    lambda tx, tw, t: w8[("v", tw)][:, 2 * t : 2 * t + 2, hsl],
                )
                with nc.allow_low_precision(reason="bf16 store"):
                    nc.vector.tensor_scalar_mul(
                        vF8[jc][:, hsl], p[:, 0:64], mxf[:, jc, 0:1]
                    )
                    nc.vector.tensor_scalar_mul(
                        vP8[jc][:, hsl], p[:, 0:64], mxf[:, jc, 1:2]
                    )

            def proj_u(pool, h, lc):
                hsl = slice(64 * h, 64 * h + 64)
                si, o = (lc * 128) // 512, (lc * 128) % 512
                p = pool.tile([128, 512], F32, tag="pp", name="pp")
                proj_mms(
                    p, 64,
                    lambda tx, tw, t: xs[(si, tx)][:, 2 * t : 2 * t + 2, o : o + 128],
                    lambda tx, tw, t: w8[("u", tw)][:, 2 * t : 2 * t + 2, hsl],
                )
                with nc.allow_low_precision(reason="bf16 store"):
                    nc.vector.tensor_scalar_mul(
                        u16[lc][:, hsl], p[:, 0:64], 1.0 / 16.0
                    )

            def scores_exp(scp, h, jc, half):
                """e tile [128 keys, 1024 queries] = exp(S/8) for (h, jc, half)."""
                ec, hh = h // 2, h % 2
                jsl = slice(jc * 128, (jc + 1) * 128)
                e8 = e8p.tile([128, 1024], BF16, tag="e8", name="e8")
                sc = scp.tile([128, 1024], F32, tag="sc", name="sc")
                for q in range(2):
                    q0 = half * 1024 + q * 512
                    nc.tensor.matmul(
                        sc[:, q * 512 : (q + 1) * 512],
                        k16[ec][64 * hh : 64 * hh + 64, jsl],
                        q16[ec][64 * hh : 64 * hh + 64, q0 : q0 + 512],
                        start=True, stop=True,
                    )
                with nc.allow_low_precision(reason="bf16 exp"):
                    nc.scalar.activation(e8, sc, EXP, scale=0.125)
                return e8

            def av_half(av, rs, h, jc, half, e8):
                hsl = slice(64 * h, 64 * h + 64)
                base = half * 8
                eD = None
                if base <= jc < base + 8:
                    eD = eDp.tile([128, 128], BF16, tag="eD", name="eD")
                    loc = jc * 128 - half * 1024
                    with nc.allow_low_precision(reason="mask mul"):
                        nc.gpsimd.tensor_mul(
                            eD, e8[:, loc : loc + 128], dm16[:, jc, :]
                        )
                for lc in range(base, base + 8):
                    loc = lc * 128 - half * 1024
                    if jc == lc:
                        lhsT, vt, mcol = eD, vF8, 2
                    elif jc < lc:
                        lhsT, vt, mcol = e8[:, loc : loc + 128], vF8, 0
                    else:
                        lhsT, vt, mcol = e8[:, loc : loc + 128], vP8, 1
                    nc.tensor.matmul(
                        av[:, lc * 64 : (lc + 1) * 64],
                        lhsT, vt[jc][:, hsl],
                        start=(jc == 0 and lc == base),
                        stop=(jc == NJ - 1 and lc == base + 7),
                    )
                    nc.tensor.matmul(
                        rs[:, (h % 2) * 16 + lc : (h % 2) * 16 + lc + 1],
                        lhsT, mc16[:, jc, mcol : mcol + 1],
                        start=(jc == 0 and half == 0 and lc == 0),
                        stop=(jc == NJ - 1 and lc == NLC - 1),
                    )

            def head_att(scp, projp, av, rs, h, pre=(), hooks=None):
                hooks = hooks or {}
                pend = []
                ui = 0
                for half in range(2):
                    for jc in range(NJ):
                        e = scores_exp(scp, h, jc, half)
                        if half == 0:
                            proj_v(projp, h, jc)
                        if ui == 0:
                            for f in pre:
                                f()
                        if len(pend) >= 2:
                            av_half(av, rs, h, *pend.pop(0))
                        for f in hooks.get(ui, ()):
                            f()
                        pend.append((jc, half, e))
                        ui += 1
                for item in pend:
                    av_half(av, rs, h, *item)

            def gate(av, rs, h):
                p = h % 2
                with nc.allow_low_precision(reason="gate"):
                    nc.vector.reciprocal(rec16[p], rs[:, p * 16 : (p + 1) * 16])
                    nc.vector.tensor_copy(avs, av)
                    for lc in range(NLC):
                        nc.vector.scalar_tensor_tensor(
                            g16[lc][:, 64 * h : 64 * h + 64],
                            avs[:, lc * 64 : (lc + 1) * 64],
                            rec16[p][:, lc : lc + 1],
                            u16[lc][:, 64 * h : 64 * h + 64],
                            MULT, MULT,
                        )

            def transposes_dma(ec):
                for lc in range(NLC):
                    nc.sync.dma_start_transpose(
                        gT16[(ec, lc)],
                        g16[lc][:, ec * 128 : (ec + 1) * 128],
                    )

            wo_alt = [0]
            osb_cur = [None]

            def wo_step(wop, ec, lc, fc, tail=False):
                """one W_o matmul + copy; every 8th step fires the quad DMA."""
                q, s = lc // 4, lc % 4
                if osb_cur[0] is None:
                    osb_cur[0] = osb.tile([128, 4, 1024], BF16, tag="osb", name="osb")
                o = osb_cur[0]
                p = wop.tile([128, 512], F32, tag="pp", name="pp")
                nc.tensor.matmul(
                    p,
                    gT16[(ec, lc)],
                    wo16[:, ec, fc * 512 : (fc + 1) * 512],
                    start=True, stop=True,
                )
                wo_alt[0] += 1
                with nc.allow_low_precision(reason="bf16 out"):
                    if tail and wo_alt[0] % 2 == 0:
                        nc.scalar.activation(
                            o[:, s, fc * 512 : (fc + 1) * 512], p, COPY
                        )
                    else:
                        nc.vector.tensor_copy(
                            o[:, s, fc * 512 : (fc + 1) * 512], p
                        )
                if s == 3 and fc == 1:
                    nc.sync.dma_start(
                        out=outr[ec][:, 4 * q : 4 * q + 4, :], in_=o
                    )
                    osb_cur[0] = None

            NU = 2 * NJ  # units per head

            def spread(jobs, lo, hi):
                """jobs: list of (cost, fn); place by cumulative cost."""
                hooks = {}
                total = sum(c for c, _ in jobs) or 1
                acc = 0
                for c, job in jobs:
                    hooks.setdefault(lo + (acc * (hi - lo)) // total, []).append(job)
                    acc += c
                return hooks

            with tc.tile_pool(name="av", bufs=1, space="PSUM") as avp, \
                 tc.tile_pool(name="rs", bufs=1, space="PSUM") as rsp:
                av = avp.tile([128, 1024], F32, tag="av", name="av")
                rs = rsp.tile([128, 32], F32, tag="rs", name="rs")

                # -------- phase 1: warmup, DMAs, h0, QK proj, U(h0) --------
                with tc.tile_pool(name="pp", bufs=3, space="PSUM") as pp, \
                     tc.tile_pool(name="sc1", bufs=1, space="PSUM") as sc1:
                    # PE warm-up chain during the initial DMA window
                    nc.vector.memset(wtmp, 0.0)
                    wp = pp.tile([128, 512], F32, tag="pp", name="pp")
                    for i in range(5):
                        nc.tensor.matmul(
                            wp, wtmp[:, 0:128], wtmp,
                            start=(i == 0), stop=(i == 4),
                        )

                    # input DMAs (x on SP queue, weights/masks on ACT queue)
                    dma_x(0, ("h",))
                    nc.scalar.dma_start(out=w8[("k", "h")], in_=wd[("k", "h")])
                    nc.scalar.dma_start(out=w8[("q", "h")], in_=wd[("q", "h")])
                    dma_x(1, ("h",))
                    dma_x(0, ("l",))
                    nc.scalar.dma_start(out=w8[("k", "l")], in_=wd[("k", "l")])
                    nc.scalar.dma_start(out=w8[("q", "l")], in_=wd[("q", "l")])
                    dma_x(1, ("l",))
                    for t in ("h", "l"):
                        nc.scalar.dma_start(out=w8[("v", t)], in_=wd[("v", t)])
                    for t in ("h", "l"):
                        nc.scalar.dma_start(out=w8[("u", t)], in_=wd[("u", t)])
                    nc.scalar.dma_start(out=dm16, in_=dm16d)
                    nc.scalar.dma_start(out=mc16, in_=mc16d)
                    nc.scalar.dma_start(out=mxf, in_=mxfd)
                    nc.scalar.dma_start(out=wo16, in_=wo16d)
                    nc.scalar.dma_start(out=ident, in_=identd)

                    proj_qk(pp, "k", 0, 0, 512)
                    proj_qk(pp, "q", 0, 0, 512)
                    proj_qk(pp, "q", 0, 512, 512)

                    jobs0 = []
                    jobs0.append((1, lambda: dma_x(2)))
                    for (c0, w) in kspans[1:2]:
                        jobs0.append((3, lambda c0=c0, w=w: proj_qk(pp, "k", 0, c0, w)))
                    jobs0.append((3, lambda: proj_qk(pp, "q", 0, 1024, 512)))
                    jobs0.append((1, lambda: dma_x(3)))
                    for (c0, w) in kspans[2:]:
                        jobs0.append((3, lambda c0=c0, w=w: proj_qk(pp, "k", 0, c0, w)))
                    jobs0.append((3, lambda: proj_qk(pp, "q", 0, 1536, 512)))
                    for lc in range(NLC):
                        jobs0.append((1, lambda lc=lc: proj_u(pp, 0, lc)))
                    for (c0, w) in kspans:
                        jobs0.append((3, lambda c0=c0, w=w: proj_qk(pp, "k", 1, c0, w)))
                    for ic in range(NIC):
                        jobs0.append((3, lambda ic=ic: proj_qk(pp, "q", 1, ic * 512, 512)))
                    head_att(sc1, pp, av, rs, 0, hooks=spread(jobs0, 1, NU))

                # -------- phase 2: h1-h3, ec0 wo --------
                with tc.tile_pool(name="sc2", bufs=2, space="PSUM") as sc2, \
                     tc.tile_pool(name="wop", bufs=1, space="PSUM") as wop:
                    jobs1 = [(1, lambda lc=lc: proj_u(wop, 1, lc)) for lc in range(NLC)]
                    head_att(sc2, wop, av, rs, 1,
                             pre=[lambda: gate(av, rs, 0)],
                             hooks=spread(jobs1, 1, NU))

                    jobs2 = [(1, lambda lc=lc: proj_u(wop, 2, lc)) for lc in range(NLC)]
                    jobs2 += [(1, lambda lc=lc, fc=fc: wo_step(wop, 0, lc, fc))
                              for lc in range(8) for fc in range(2)]
                    head_att(sc2, wop, av, rs, 2,
                             pre=[lambda: gate(av, rs, 1), lambda: transposes_dma(0)],
                             hooks=spread(jobs2, 1, NU))

                    jobs3 = [(1, lambda lc=lc: proj_u(wop, 3, lc)) for lc in range(NLC)]
                    jobs3 += [(1, lambda lc=lc, fc=fc: wo_step(wop, 0, lc, fc))
                              for lc in range(8, NLC) for fc in range(2)]
                    head_att(sc2, wop, av, rs, 3,
                             pre=[lambda: gate(av, rs, 2)],
                             hooks=spread(jobs3, 1, NU))
                    gate(av, rs, 3)

            # -------- phase 3: tail (av/rs closed): PE transposes + ec1 wo --------
            with tc.tile_pool(name="wo2", bufs=3, space="PSUM") as wo2, \
                 tc.tile_pool(name="tp", bufs=2, space="PSUM") as tpp:
                def tail_tp(lc):
                    t = tpp.tile([128, 128], BF16, tag="tp", name="tp")
                    nc.tensor.transpose(t, g16[lc][:, 128:256], ident)
                    with nc.allow_low_precision(reason="bf16 transpose"):
                        if lc % 2 == 0:
                            nc.vector.tensor_copy(gT16[(1, lc)], t)
                        else:
                            nc.scalar.activation(gT16[(1, lc)], t, COPY)

                def tail_wo(lc):
                    q, s = lc // 4, lc % 4
                    if osb_cur[0] is None:
                        osb_cur[0] = osb.tile([128, 4, 1024], BF16, tag="osb", name="osb")
                    o = osb_cur[0]
                    p = wo2.tile([128, 1024], F32, tag="wq", name="wq")
                    for fc in range(2):
                        nc.tensor.matmul(
                            p[:, fc * 512 : (fc + 1) * 512],
                            gT16[(1, lc)],
                            wo16[:, 1, fc * 512 : (fc + 1) * 512],
                            start=True, stop=True,
                        )
                    with nc.allow_low_precision(reason="bf16 out"):
                        if lc % 2 == 0:
                            nc.scalar.activation(o[:, s, :], p, COPY)
                        else:
                            nc.vector.tensor_copy(o[:, s, :], p)
                    if s == 3:
                        nc.sync.dma_start(
                            out=outr[1][:, 4 * q : 4 * q + 4, :], in_=o
                        )
                        osb_cur[0] = None

                tail_tp(0)
                tail_tp(1)
                for lc in range(NLC):
                    if lc + 2 < NLC:
                        tail_tp(lc + 2)
                    tail_wo(lc)

    nc.compile()
    return nc


def _hilo(a):
    f8 = ml_dtypes.float8_e4m3
    hi = a.astype(f8)
    lo = (a - hi.astype(np.float32)).astype(f8)
    return hi, lo


def _host_inputs(NJ, x, token_types, seq_lens, W_q, W_k, W_v, W_u, W_o):
    x = np.asarray(x, dtype=np.float32)
    token_types = np.asarray(token_types)
    seq_lens = np.asarray(seq_lens)
    W = {
        "q": np.asarray(W_q, dtype=np.float32),
        "k": np.asarray(W_k, dtype=np.float32),
        "v": np.asarray(W_v, dtype=np.float32),
        "u": np.asarray(W_u, dtype=np.float32),
    }
    W_o = np.asarray(W_o, dtype=np.float32)
    bf = ml_dtypes.bfloat16

    per_batch = []
    for b in range(B):
        xt = np.ascontiguousarray(x[b].T.reshape(NDC, 128, L).transpose(1, 0, 2))
        xh, xl = _hilo(xt)
        prompt = np.asarray(token_types[b] < 3)
        valid = np.arange(L) < int(seq_lens[b])
        dm16 = np.zeros((128, NJ, 128), bf)
        mc16 = np.zeros((128, NJ, 3), bf)
        mxf = np.zeros((128, NJ, 2), np.float32)
        for jc in range(NJ):
            j = np.arange(jc * 128, (jc + 1) * 128)
            i = j  # true-diagonal block
            allow = valid[j][:, None] & (prompt[j][:, None] | (j[:, None] <= i[None, :]))
            dm16[:, jc, :] = allow.astype(np.float32)
            mF = valid[j].astype(np.float32)
            mP = (valid[j] & prompt[j]).astype(np.float32)
            mc16[:, jc, 0] = mF
            mc16[:, jc, 1] = mP
            mc16[:, jc, 2] = 1.0
            mxf[:, jc, 0] = mF / 16.0
            mxf[:, jc, 1] = mP / 16.0
        per_batch.append((xh, xl, dm16, mc16, mxf))

    ident = np.eye(128, dtype=bf)
    in_maps = []
    for c in range(8):
        b, gi = c // 4, c % 4
        e0 = E * gi
        xh, xl, dm16, mc16, mxf = per_batch[b]
        m = {"x_h": xh, "x_l": xl, "dm16": dm16, "mc16": mc16, "mxf": mxf,
             "ident": ident}
        for nm in ("q", "k", "v", "u"):
            wt = np.ascontiguousarray(
                (W[nm][e0 : e0 + E] * 16.0).T.reshape(NDC, 128, E).transpose(1, 0, 2)
            )
            wh, wl = _hilo(wt)
            m[f"w{nm}_h"] = wh
            m[f"w{nm}_l"] = wl
        m["wo16"] = np.ascontiguousarray(
            W_o[:, e0 : e0 + E].T.reshape(2, 128, D).transpose(1, 0, 2)
        ).astype(bf)
        in_maps.append(m)
    return in_maps


def kernel(x, token_types, seq_lens, W_q, W_k, W_v, W_u, W_o, **_run_kwargs):
    seq = np.asarray(seq_lens)
    NJ = int(np.ceil(seq.max() / 128.0))
    NJ = max(1, min(NLC, NJ))
    if ("nc", NJ) not in _cache:
        _cache[("nc", NJ)] = _build(NJ)
    nc = _cache[("nc", NJ)]
    in_maps = _host_inputs(NJ, x, token_types, seq_lens, W_q, W_k, W_v, W_u, W_o)
    try:
        res = run_bass_kernel_spmd(nc, in_maps, list(range(8)), **_run_kwargs)
    except Exception as ex:  # transient NRT device wedge: retry once
        if "UNRECOVERABLE" not in str(ex) and "UNAVAILABLE" not in str(ex):
            raise
        res = run_bass_kernel_spmd(nc, in_maps, list(range(8)), **_run_kwargs)
    _cache["last_result"] = res
    _cache["nc"] = nc  # for test.py TimelineSim
    full = np.zeros((B, L, D), np.float64)
    for c in range(8):
        r = res.results[c]
        full[c // 4] += r["out0"].astype(np.float64) + r["out1"].astype(np.float64)
    return full.astype(np.float32)


# revision 12
# speedup vs baseline: 1.3327x; 1.0023x over previous
"""HSTU attention (B=2, L=2048, D=1024, H=16) on 8 TRN2 NeuronCores.

Sharding: batch (2) x head-group (4 heads, 256 features) -> 8 cores.

Per core, for its batch b and 4 heads:
  - Projections run as 3-term fp8 DoubleRow matmuls: x and 16*W are sent as
    fp8 (hi) plus fp8 residual (lo); psum accumulates hi*hi + hi*lo + lo*hi
    (the dropped lo*lo term is ~1e-3 relative).  0.75x the cycles of bf16
    at bf16-class accuracy; the 1/16 is folded into the psum->SBUF copies.
  - Scores S^T = K^T.T @ Q in bf16, [keys x queries] layout, psum tiles of
    [128, 1024] (2 banks); exp(S/8) on ACT (scale=0.125) -> bf16 e tiles.
  - Key chunks beyond max(seq_len) are skipped (runtime-specialized NJ);
    masking is folded into the AV operands: V is premasked into vF (valid)
    and vP (prompt&valid), true-diagonal 128x128 blocks get a {0,1} mask
    multiply (Pool engine), row sums use mask columns.
  - AV is swapped: out[tokens, feats] += e_chunk.T @ v (N=64), with N=1
    row-sum matmuls into a shared psum bank; softmax normalization + U
    gating is a per-partition scalar_tensor_tensor from an SBUF copy.
  - g is transposed per 128x128 chunk: DMA xbar transpose for the first
    half (ec0, mid-kernel), PE transpose via identity for the tail half.
  - W_o partials per ec-half in bf16; outputs land in two bf16 partial
    tensors, DMA'd four token-chunks at a time.
Host sums the 8 partial outputs per batch.

Scheduling: a software-pipelined (jc, query-half) unit loop per head with
hooks spreading projections / W_o groups into PE slack; per-chunk SBUF
tiles avoid false tile-granularity dependencies; a warm-up matmul chain
brings the PE out of its low p-state during the initial DMA window.
"""

import sys

for _p in ("/opt/trn_rl_repo", "/root/.axon_site/_ro/trn_rl_repo"):
    if _p not in sys.path:
        sys.path.insert(0, _p)

import numpy as np
import ml_dtypes

import concourse.bass as bass  # noqa: F401
import concourse.mybir as mybir
import concourse.tile as tile
from concourse import bacc
from concourse.bass_utils import run_bass_kernel_spmd

F32 = mybir.dt.float32
BF16 = mybir.dt.bfloat16
F8 = mybir.dt.float8e4
EXP = mybir.ActivationFunctionType.Exp
COPY = mybir.ActivationFunctionType.Copy
DR = mybir.MatmulPerfMode.DoubleRow
MULT = mybir.AluOpType.mult

B, L, D, H = 2, 2048, 1024, 16
DK = D // H          # 64
HPC = 4              # heads per core
E = HPC * DK         # 256 features per core
NDC = D // 128       # 8 contraction chunks for projections
NLC = L // 128       # 16 token chunks
NIC = L // 512       # 4 token 512-spans

_cache = {}


def _build(NJ):
    NLK = NJ * 128
    kspans = [(s, min(512, NLK - s)) for s in range(0, NLK, 512)]

    nc = bacc.Bacc("TRN2", target_bir_lowering=False, debug=False)

    xd = {
        t: nc.dram_tensor(f"x_{t}", [128, NDC, L], F8, kind="ExternalInput").ap()
        for t in ("h", "l")
    }
    wd = {
        (nm, t): nc.dram_tensor(f"w{nm}_{t}", [128, NDC, E], F8, kind="ExternalInput").ap()
        for nm in ("q", "k", "v", "u") for t in ("h", "l")
    }
    wo16d = nc.dram_tensor("wo16", [128, 2, D], BF16, kind="ExternalInput").ap()
    dm16d = nc.dram_tensor("dm16", [128, NJ, 128], BF16, kind="ExternalInput").ap()
    mc16d = nc.dram_tensor("mc16", [128, NJ, 3], BF16, kind="ExternalInput").ap()
    mxfd = nc.dram_tensor("mxf", [128, NJ, 2], F32, kind="ExternalInput").ap()
    identd = nc.dram_tensor("ident", [128, 128], BF16, kind="ExternalInput").ap()
    outd = [
        nc.dram_tensor(f"out{ec}", [L, D], BF16, kind="ExternalOutput").ap()
        for ec in range(2)
    ]
    # out viewed as [tok-in-chunk 128, chunk 16, feat 1024] for merged DMAs
    outr = [o.rearrange("(a p) d -> p a d", p=128) for o in outd]

    with tile.TileContext(nc) as tc:
        with tc.tile_pool(name="persist", bufs=1) as persist, \
             tc.tile_pool(name="e8p", bufs=4) as e8p, \
             tc.tile_pool(name="eDp", bufs=4) as eDp, \
             tc.tile_pool(name="osb", bufs=2) as osb:
            xs = {
                (s, t): persist.tile([128, NDC, 512], F8, tag=f"xs{s}{t}", name=f"xs{s}{t}")
                for s in range(NIC) for t in ("h", "l")
            }
            w8 = {
                k: persist.tile([128, NDC, E], F8, tag=f"w{k[0]}{k[1]}", name=f"w{k[0]}{k[1]}")
                for k in wd
            }
            wo16 = persist.tile([128, 2, D], BF16, tag="wo16", name="wo16")
            dm16 = persist.tile([128, NJ, 128], BF16, tag="dm16", name="dm16")
            mc16 = persist.tile([128, NJ, 3], BF16, tag="mc16", name="mc16")
            mxf = persist.tile([128, NJ, 2], F32, tag="mxf", name="mxf")
            ident = persist.tile([128, 128], BF16, tag="ident", name="ident")
            wtmp = persist.tile([128, 512], BF16, tag="wtmp", name="wtmp")
            q16 = [persist.tile([128, L], BF16, tag=f"q16_{ec}", name=f"q16_{ec}")
                   for ec in range(2)]
            k16 = [persist.tile([128, NLK], BF16, tag=f"k16_{ec}", name=f"k16_{ec}")
                   for ec in range(2)]
            u16 = [persist.tile([128, E], BF16, tag=f"u16_{lc}", name=f"u16_{lc}")
                   for lc in range(NLC)]
            vF8 = [persist.tile([128, E], BF16, tag=f"vF_{jc}", name=f"vF_{jc}")
                   for jc in range(NJ)]
            vP8 = [persist.tile([128, E], BF16, tag=f"vP_{jc}", name=f"vP_{jc}")
                   for jc in range(NJ)]
            g16 = [persist.tile([128, E], BF16, tag=f"g_{lc}", name=f"g_{lc}")
                   for lc in range(NLC)]
            gT16 = {(ec, lc): persist.tile([128, 128], BF16, tag=f"gt{ec}_{lc}", name=f"gt{ec}_{lc}")
                    for ec in range(2) for lc in range(NLC)}
            avs = persist.tile([128, 1024], F32, tag="avs", name="avs")
            rec16 = [persist.tile([128, 16], F32, tag=f"rec{p}", name=f"rec{p}")
                     for p in range(2)]

            # -------- emission helpers --------
            def dma_x(si, which=("h", "l")):
                s0 = si * 512
                for t in which:
                    nc.sync.dma_start(out=xs[(si, t)], in_=xd[t][:, :, s0 : s0 + 512])

            def proj_mms(p, w, lhs_of, rhs_of):
                """3-term hi/lo DR accumulation into psum slice p[:, 0:w]."""
                terms = (("h", "h"), ("h", "l"), ("l", "h"))
                n = NDC // 2
                first = True
                for (tx, tw) in terms:
                    for t in range(n):
                        nc.tensor.matmul(
                            p[:, 0:w],
                            lhs_of(tx, tw, t),
                            rhs_of(tx, tw, t),
                            start=first,
                            stop=(tx, tw) == ("l", "h") and t == n - 1,
                            perf_mode=DR,
                        )
                        first = False

            def proj_qk(pool, nm, ec, c0, w):
                """q16/k16[ec][:, c0:c0+w] = (x @ (16W).T)/16 in [feat, tok]."""
                p = pool.tile([128, 512], F32, tag="pp", name="pp")
                si, o = c0 // 512, c0 % 512
                proj_mms(
                    p, w,
                    lambda tx, tw, t: w8[(nm, tw)][:, 2 * t : 2 * t + 2, ec * 128 : (ec + 1) * 128],
                    lambda tx, tw, t: xs[(si, tx)][:, 2 * t : 2 * t + 2, o : o + w],
                )
                dest = q16 if nm == "q" else k16
                with nc.allow_low_precision(reason="bf16 store"):
                    nc.vector.tensor_scalar_mul(
                        dest[ec][:, c0 : c0 + w], p[:, 0:w], 1.0 / 16.0
                    )

            def proj_v(pool, h, jc):
                hsl = slice(64 * h, 64 * h + 64)
                si, o = (jc * 128) // 512, (jc * 128) % 512
                p = pool.tile([128, 512], F32, tag="pp", name="pp")
                proj_mms(
                    p, 64,
                    lambda tx, tw, t: xs[(si, tx)][:, 2 * t : 2 * t + 2, o : o + 128],
                    lambda tx, tw, t: w8[("v", tw)][:, 2 * t : 2 * t + 2, hsl],
                )
                with nc.allow_low_precision(reason="bf16 store"):
                    nc.vector.tensor_scalar_mul(
                        vF8[jc][:, hsl], p[:, 0:64], mxf[:, jc, 0:1]
                    )
                    nc.vector.tensor_scalar_mul(
                        vP8[jc][:, hsl], p[:, 0:64], mxf[:, jc, 1:2]
                    )

            def proj_u(pool, h, lc):
                hsl = slice(64 * h, 64 * h + 64)
                si, o = (lc * 128) // 512, (lc * 128) % 512
                p = pool.tile([128, 512], F32, tag="pp", name="pp")
                proj_mms(
                    p, 64,
                    lambda tx, tw, t: xs[(si, tx)][:, 2 * t : 2 * t + 2, o : o + 128],
                    lambda tx, tw, t: w8[("u", tw)][:, 2 * t : 2 * t + 2, hsl],
                )
                with nc.allow_low_precision(reason="bf16 store"):
                    nc.vector.tensor_scalar_mul(
                        u16[lc][:, hsl], p[:, 0:64], 1.0 / 16.0
                    )

            def scores_exp(scp, h, jc, half):
                """e tile [128 keys, 1024 queries] = exp(S/8) for (h, jc, half).
                Also precomputes the diag-masked eD tile when this (jc, half)
                contains the true-diagonal block."""
                ec, hh = h // 2, h % 2
                jsl = slice(jc * 128, (jc + 1) * 128)
                e8 = e8p.tile([128, 1024], BF16, tag="e8", name="e8")
                sc = scp.tile([128, 1024], F32, tag="sc", name="sc")
                for q in range(2):
                    q0 = half * 1024 + q * 512
                    nc.tensor.matmul(
                        sc[:, q * 512 : (q + 1) * 512],
                        k16[ec][64 * hh : 64 * hh + 64, jsl],
                        q16[ec][64 * hh : 64 * hh + 64, q0 : q0 + 512],
                        start=True, stop=True,
                    )
                with nc.allow_low_precision(reason="bf16 exp"):
                    nc.scalar.activation(e8, sc, EXP, scale=0.125)
                eD = None
                if half * 8 <= jc < half * 8 + 8:
                    eD = eDp.tile([128, 128], BF16, tag="eD", name="eD")
                    loc = jc * 128 - half * 1024
                    with nc.allow_low_precision(reason="mask mul"):
                        nc.gpsimd.tensor_mul(
                            eD, e8[:, loc : loc + 128], dm16[:, jc, :]
                        )
                return e8, eD

            def av_half(av, rs, h, jc, half, e8, eD):
                hsl = slice(64 * h, 64 * h + 64)
                base = half * 8
                for lc in range(base, base + 8):
                    loc = lc * 128 - half * 1024
                    if jc == lc:
                        lhsT, vt, mcol = eD, vF8, 2
                    elif jc < lc:
                        lhsT, vt, mcol = e8[:, loc : loc + 128], vF8, 0
                    else:
                        lhsT, vt, mcol = e8[:, loc : loc + 128], vP8, 1
                    nc.tensor.matmul(
                        av[:, lc * 64 : (lc + 1) * 64],
                        lhsT, vt[jc][:, hsl],
                        start=(jc == 0 and lc == base),
                        stop=(jc == NJ - 1 and lc == base + 7),
                    )
                    nc.tensor.matmul(
                        rs[:, (h % 2) * 16 + lc : (h % 2) * 16 + lc + 1],
                        lhsT, mc16[:, jc, mcol : mcol + 1],
                        start=(jc == 0 and half == 0 and lc == 0),
                        stop=(jc == NJ - 1 and lc == NLC - 1),
                    )

            def head_att(scp, projp, av, rs, h, pre=(), hooks=None):
                hooks = hooks or {}
                pend = []
                ui = 0
                for half in range(2):
                    for jc in range(NJ):
                        e, eD = scores_exp(scp, h, jc, half)
                        if half == 0:
                            proj_v(projp, h, jc)
                        if ui == 0:
                            for f in pre:
                                f()
                        if len(pend) >= 2:
                            av_half(av, rs, h, *pend.pop(0))
                        for f in hooks.get(ui, ()):
                            f()
                        pend.append((jc, half, e, eD))
                        ui += 1
                for item in pend:
                    av_half(av, rs, h, *item)

            def gate(av, rs, h):
                p = h % 2
                with nc.allow_low_precision(reason="gate"):
                    nc.vector.reciprocal(rec16[p], rs[:, p * 16 : (p + 1) * 16])
                    nc.vector.tensor_copy(avs, av)
                    for lc in range(NLC):
                        nc.vector.scalar_tensor_tensor(
                            g16[lc][:, 64 * h : 64 * h + 64],
                            avs[:, lc * 64 : (lc + 1) * 64],
                            rec16[p][:, lc : lc + 1],
                            u16[lc][:, 64 * h : 64 * h + 64],
                            MULT, MULT,
                        )

            def transposes_dma(ec):
                for lc in range(NLC):
                    nc.sync.dma_start_transpose(
                        gT16[(ec, lc)],
                        g16[lc][:, ec * 128 : (ec + 1) * 128],
                    )

            wo_alt = [0]
            osb_cur = [None]

            def wo_step(wop, ec, lc, fc, tail=False):
                """one W_o matmul + copy; every 8th step fires the quad DMA."""
                q, s = lc // 4, lc % 4
                if osb_cur[0] is None:
                    osb_cur[0] = osb.tile([128, 4, 1024], BF16, tag="osb", name="osb")
                o = osb_cur[0]
                p = wop.tile([128, 512], F32, tag="pp", name="pp")
                nc.tensor.matmul(
                    p,
                    gT16[(ec, lc)],
                    wo16[:, ec, fc * 512 : (fc + 1) * 512],
                    start=True, stop=True,
                )
                wo_alt[0] += 1
                with nc.allow_low_precision(reason="bf16 out"):
                    if tail and wo_alt[0] % 2 == 0:
                        nc.scalar.activation(
                            o[:, s, fc * 512 : (fc + 1) * 512], p, COPY
                        )
                    else:
                        nc.vector.tensor_copy(
                            o[:, s, fc * 512 : (fc + 1) * 512], p
                        )
                if s == 3 and fc == 1:
                    nc.sync.dma_start(
                        out=outr[ec][:, 4 * q : 4 * q + 4, :], in_=o
                    )
                    osb_cur[0] = None

            NU = 2 * NJ  # units per head

            def spread(jobs, lo, hi):
                """jobs: list of (cost, fn); place by cumulative cost."""
                hooks = {}
                total = sum(c for c, _ in jobs) or 1
                acc = 0
                for c, job in jobs:
                    hooks.setdefault(lo + (acc * (hi - lo)) // total, []).append(job)
                    acc += c
                return hooks

            with tc.tile_pool(name="av", bufs=1, space="PSUM") as avp, \
                 tc.tile_pool(name="rs", bufs=1, space="PSUM") as rsp:
                av = avp.tile([128, 1024], F32, tag="av", name="av")
                rs = rsp.tile([128, 32], F32, tag="rs", name="rs")

                # -------- phase 1: warmup, DMAs, h0, QK proj, U(h0) --------
                with tc.tile_pool(name="pp", bufs=3, space="PSUM") as pp, \
                     tc.tile_pool(name="sc1", bufs=1, space="PSUM") as sc1:
                    # PE warm-up chain during the initial DMA window
                    nc.vector.memset(wtmp, 0.0)
                    wp = pp.tile([128, 512], F32, tag="pp", name="pp")
                    for i in range(5):
                        nc.tensor.matmul(
                            wp, wtmp[:, 0:128], wtmp,
                            start=(i == 0), stop=(i == 4),
                        )

                    # input DMAs (x on SP queue, weights/masks on ACT queue)
                    dma_x(0, ("h",))
                    nc.scalar.dma_start(out=w8[("k", "h")], in_=wd[("k", "h")])
                    nc.scalar.dma_start(out=w8[("q", "h")], in_=wd[("q", "h")])
                    dma_x(1, ("h",))
                    dma_x(0, ("l",))
                    nc.scalar.dma_start(out=w8[("k", "l")], in_=wd[("k", "l")])
                    nc.scalar.dma_start(out=w8[("q", "l")], in_=wd[("q", "l")])
                    dma_x(1, ("l",))
                    for t in ("h", "l"):
                        nc.scalar.dma_start(out=w8[("v", t)], in_=wd[("v", t)])
                    for t in ("h", "l"):
                        nc.scalar.dma_start(out=w8[("u", t)], in_=wd[("u", t)])
                    nc.scalar.dma_start(out=dm16, in_=dm16d)
                    nc.scalar.dma_start(out=mc16, in_=mc16d)
                    nc.scalar.dma_start(out=mxf, in_=mxfd)
                    nc.scalar.dma_start(out=wo16, in_=wo16d)
                    nc.scalar.dma_start(out=ident, in_=identd)

                    proj_qk(pp, "k", 0, 0, 512)
                    proj_qk(pp, "q", 0, 0, 512)
                    proj_qk(pp, "q", 0, 512, 512)

                    jobs0 = []
                    jobs0.append((1, lambda: dma_x(2)))
                    for (c0, w) in kspans[1:2]:
                        jobs0.append((3, lambda c0=c0, w=w: proj_qk(pp, "k", 0, c0, w)))
                    jobs0.append((3, lambda: proj_qk(pp, "q", 0, 1024, 512)))
                    jobs0.append((1, lambda: dma_x(3)))
                    for (c0, w) in kspans[2:]:
                        jobs0.append((3, lambda c0=c0, w=w: proj_qk(pp, "k", 0, c0, w)))
                    jobs0.append((3, lambda: proj_qk(pp, "q", 0, 1536, 512)))
                    for lc in range(NLC):
                        jobs0.append((1, lambda lc=lc: proj_u(pp, 0, lc)))
                    for (c0, w) in kspans:
                        jobs0.append((3, lambda c0=c0, w=w: proj_qk(pp, "k", 1, c0, w)))
                    for ic in range(NIC):
                        jobs0.append((3, lambda ic=ic: proj_qk(pp, "q", 1, ic * 512, 512)))
                    head_att(sc1, pp, av, rs, 0, hooks=spread(jobs0, 1, NU))

                # -------- phase 2: h1-h3, ec0 wo --------
                with tc.tile_pool(name="sc2", bufs=2, space="PSUM") as sc2, \
                     tc.tile_pool(name="wop", bufs=1, space="PSUM") as wop:
                    jobs1 = [(1, lambda lc=lc: proj_u(wop, 1, lc)) for lc in range(NLC)]
                    head_att(sc2, wop, av, rs, 1,
                             pre=[lambda: gate(av, rs, 0)],
                             hooks=spread(jobs1, 1, NU))

                    jobs2 = [(1, lambda lc=lc: proj_u(wop, 2, lc)) for lc in range(NLC)]
                    jobs2 += [(1, lambda lc=lc, fc=fc: wo_step(wop, 0, lc, fc))
                              for lc in range(8) for fc in range(2)]
                    head_att(sc2, wop, av, rs, 2,
                             pre=[lambda: gate(av, rs, 1), lambda: transposes_dma(0)],
                             hooks=spread(jobs2, 1, NU))

                    jobs3 = [(1, lambda lc=lc: proj_u(wop, 3, lc)) for lc in range(NLC)]
                    jobs3 += [(1, lambda lc=lc, fc=fc: wo_step(wop, 0, lc, fc))
                              for lc in range(8, NLC) for fc in range(2)]
                    head_att(sc2, wop, av, rs, 3,
                             pre=[lambda: gate(av, rs, 2)],
                             hooks=spread(jobs3, 1, NU))
                    gate(av, rs, 3)

            # -------- phase 3: tail (av/rs closed): PE transposes + ec1 wo --------
            with tc.tile_pool(name="wo2", bufs=3, space="PSUM") as wo2, \
                 tc.tile_pool(name="tp", bufs=2, space="PSUM") as tpp:
                def tail_tp(lc):
                    t = tpp.tile([128, 128], BF16, tag="tp", name="tp")
                    nc.tensor.transpose(t, g16[lc][:, 128:256], ident)
                    with nc.allow_low_precision(reason="bf16 transpose"):
                        if lc % 2 == 0:
                            nc.vector.tensor_copy(gT16[(1, lc)], t)
                        else:
                            nc.scalar.activation(gT16[(1, lc)], t, COPY)

                def tail_wo(lc):
                    q, s = lc // 4, lc % 4
                    if osb_cur[0] is None:
                        osb_cur[0] = osb.tile([128, 4, 1024], BF16, tag="osb", name="osb")
                    o = osb_cur[0]
                    p = wo2.tile([128, 1024], F32, tag="wq", name="wq")
                    for fc in range(2):
                        nc.tensor.matmul(
                            p[:, fc * 512 : (fc + 1) * 512],
                            gT16[(1, lc)],
                            wo16[:, 1, fc * 512 : (fc + 1) * 512],
                            start=True, stop=True,
                        )
                    with nc.allow_low_precision(reason="bf16 out"):
                        if lc % 2 == 0:
                            nc.scalar.activation(o[:, s, :], p, COPY)
                        else:
                            nc.vector.tensor_copy(o[:, s, :], p)
                    if s == 3:
                        nc.sync.dma_start(
                            out=outr[1][:, 4 * q : 4 * q + 4, :], in_=o
                        )
                        osb_cur[0] = None

                tail_tp(0)
                tail_tp(1)
                for lc in range(NLC):
                    if lc + 2 < NLC:
                        tail_tp(lc + 2)
                    tail_wo(lc)

    nc.compile()
    return nc


def _hilo(a):
    f8 = ml_dtypes.float8_e4m3
    hi = a.astype(f8)
    lo = (a - hi.astype(np.float32)).astype(f8)
    return hi, lo


def _host_inputs(NJ, x, token_types, seq_lens, W_q, W_k, W_v, W_u, W_o):
    x = np.asarray(x, dtype=np.float32)
    token_types = np.asarray(token_types)
    seq_lens = np.asarray(seq_lens)
    W = {
        "q": np.asarray(W_q, dtype=np.float32),
        "k": np.asarray(W_k, dtype=np.float32),
        "v": np.asarray(W_v, dtype=np.float32),
        "u": np.asarray(W_u, dtype=np.float32),
    }
    W_o = np.asarray(W_o, dtype=np.float32)
    bf = ml_dtypes.bfloat16

    per_batch = []
    for b in range(B):
        xt = np.ascontiguousarray(x[b].T.reshape(NDC, 128, L).transpose(1, 0, 2))
        xh, xl = _hilo(xt)
        prompt = np.asarray(token_types[b] < 3)
        valid = np.arange(L) < int(seq_lens[b])
        dm16 = np.zeros((128, NJ, 128), bf)
        mc16 = np.zeros((128, NJ, 3), bf)
        mxf = np.zeros((128, NJ, 2), np.float32)
        for jc in range(NJ):
            j = np.arange(jc * 128, (jc + 1) * 128)
            i = j  # true-diagonal block
            allow = valid[j][:, None] & (prompt[j][:, None] | (j[:, None] <= i[None, :]))
            dm16[:, jc, :] = allow.astype(np.float32)
            mF = valid[j].astype(np.float32)
            mP = (valid[j] & prompt[j]).astype(np.float32)
            mc16[:, jc, 0] = mF
            mc16[:, jc, 1] = mP
            mc16[:, jc, 2] = 1.0
            mxf[:, jc, 0] = mF / 16.0
            mxf[:, jc, 1] = mP / 16.0
        per_batch.append((xh, xl, dm16, mc16, mxf))

    ident = np.eye(128, dtype=bf)
    in_maps = []
    for c in range(8):
        b, gi = c // 4, c % 4
        e0 = E * gi
        xh, xl, dm16, mc16, mxf = per_batch[b]
        m = {"x_h": xh, "x_l": xl, "dm16": dm16, "mc16": mc16, "mxf": mxf,
             "ident": ident}
        for nm in ("q", "k", "v", "u"):
            wt = np.ascontiguousarray(
                (W[nm][e0 : e0 + E] * 16.0).T.reshape(NDC, 128, E).transpose(1, 0, 2)
            )
            wh, wl = _hilo(wt)
            m[f"w{nm}_h"] = wh
            m[f"w{nm}_l"] = wl
        m["wo16"] = np.ascontiguousarray(
            W_o[:, e0 : e0 + E].T.reshape(2, 128, D).transpose(1, 0, 2)
        ).astype(bf)
        in_maps.append(m)
    return in_maps


def kernel(x, token_types, seq_lens, W_q, W_k, W_v, W_u, W_o, **_run_kwargs):
    seq = np.asarray(seq_lens)
    NJ = int(np.ceil(seq.max() / 128.0))
    NJ = max(1, min(NLC, NJ))
    if ("nc", NJ) not in _cache:
        _cache[("nc", NJ)] = _build(NJ)
    nc = _cache[("nc", NJ)]
    in_maps = _host_inputs(NJ, x, token_types, seq_lens, W_q, W_k, W_v, W_u, W_o)
    try:
        res = run_bass_kernel_spmd(nc, in_maps, list(range(8)), **_run_kwargs)
    except Exception as ex:  # transient NRT device wedge: retry once
        if "UNRECOVERABLE" not in str(ex) and "UNAVAILABLE" not in str(ex):
            raise
        res = run_bass_kernel_spmd(nc, in_maps, list(range(8)), **_run_kwargs)
    _cache["last_result"] = res
    _cache["nc"] = nc  # for test.py TimelineSim
    full = np.zeros((B, L, D), np.float64)
    for c in range(8):
        r = res.results[c]
        full[c // 4] += r["out0"].astype(np.float64) + r["out1"].astype(np.float64)
    return full.astype(np.float32)


# revision 13
# speedup vs baseline: 1.3509x; 1.0137x over previous
"""HSTU attention (B=2, L=2048, D=1024, H=16) on 8 TRN2 NeuronCores.

Sharding: batch (2) x head-group (4 heads, 256 features) -> 8 cores.

Per core, for its batch b and 4 heads:
  - Projections run as 3-term fp8 DoubleRow matmuls: x and 16*W are sent as
    fp8 (hi) plus fp8 residual (lo); psum accumulates hi*hi + hi*lo + lo*hi
    (the dropped lo*lo term is ~1e-3 relative).  0.75x the cycles of bf16
    at bf16-class accuracy; the 1/16 is folded into the psum->SBUF copies.
  - Scores S^T = K^T.T @ Q in bf16, [keys x queries] layout, psum tiles of
    [128, 1024] (2 banks); exp(S/8) on ACT (scale=0.125) -> bf16 e tiles.
  - Key chunks beyond max(seq_len) are skipped (runtime-specialized NJ);
    masking is folded into the AV operands: V is premasked into vF (valid)
    and vP (prompt&valid), true-diagonal 128x128 blocks get a {0,1} mask
    multiply (Pool engine), row sums use mask columns.
  - AV is swapped: out[tokens, feats] += e_chunk.T @ v (N=64), with N=1
    row-sum matmuls into a shared psum bank; softmax normalization + U
    gating is a per-partition scalar_tensor_tensor from an SBUF copy.
  - g is transposed per 128x128 chunk: DMA xbar transpose for the first
    half (ec0, mid-kernel), PE transpose via identity for the tail half.
  - W_o partials per ec-half in bf16; outputs land in two bf16 partial
    tensors, DMA'd four token-chunks at a time.
Host sums the 8 partial outputs per batch.

Scheduling: a software-pipelined (jc, query-half) unit loop per head with
hooks spreading projections / W_o groups into PE slack; per-chunk SBUF
tiles avoid false tile-granularity dependencies; a warm-up matmul chain
brings the PE out of its low p-state during the initial DMA window.
"""

import sys

for _p in ("/opt/trn_rl_repo", "/root/.axon_site/_ro/trn_rl_repo"):
    if _p not in sys.path:
        sys.path.insert(0, _p)

import numpy as np
import ml_dtypes

import concourse.bass as bass  # noqa: F401
import concourse.mybir as mybir
import concourse.tile as tile
from concourse import bacc
from concourse.bass_utils import run_bass_kernel_spmd

F32 = mybir.dt.float32
BF16 = mybir.dt.bfloat16
F8 = mybir.dt.float8e4
EXP = mybir.ActivationFunctionType.Exp
COPY = mybir.ActivationFunctionType.Copy
DR = mybir.MatmulPerfMode.DoubleRow
MULT = mybir.AluOpType.mult

B, L, D, H = 2, 2048, 1024, 16
DK = D // H          # 64
HPC = 4              # heads per core
E = HPC * DK         # 256 features per core
NDC = D // 128       # 8 contraction chunks for projections
NLC = L // 128       # 16 token chunks
NIC = L // 512       # 4 token 512-spans

_cache = {}


def _build(NJ):
    NLK = NJ * 128
    kspans = [(s, min(512, NLK - s)) for s in range(0, NLK, 512)]

    nc = bacc.Bacc("TRN2", target_bir_lowering=False, debug=False)

    xd = {
        t: nc.dram_tensor(f"x_{t}", [128, NDC, L], F8, kind="ExternalInput").ap()
        for t in ("h", "l")
    }
    wd = {
        (nm, t): nc.dram_tensor(f"w{nm}_{t}", [128, NDC, E], F8, kind="ExternalInput").ap()
        for nm in ("q", "k", "v", "u") for t in ("h", "l")
    }
    wo16d = nc.dram_tensor("wo16", [128, 2, D], BF16, kind="ExternalInput").ap()
    dm16d = nc.dram_tensor("dm16", [128, NJ, 128], BF16, kind="ExternalInput").ap()
    mc16d = nc.dram_tensor("mc16", [128, NJ, 3], BF16, kind="ExternalInput").ap()
    mxfd = nc.dram_tensor("mxf", [128, NJ, 2], F32, kind="ExternalInput").ap()
    identd = nc.dram_tensor("ident", [128, 128], BF16, kind="ExternalInput").ap()
    outd = [
        nc.dram_tensor(f"out{ec}", [L, D], BF16, kind="ExternalOutput").ap()
        for ec in range(2)
    ]
    # out viewed as [tok-in-chunk 128, chunk 16, feat 1024] for merged DMAs
    outr = [o.rearrange("(a p) d -> p a d", p=128) for o in outd]

    with tile.TileContext(nc) as tc:
        with tc.tile_pool(name="persist", bufs=1) as persist, \
             tc.tile_pool(name="e8p", bufs=5) as e8p, \
             tc.tile_pool(name="eDp", bufs=5) as eDp, \
             tc.tile_pool(name="osb", bufs=2) as osb:
            xs = {
                (s, t): persist.tile([128, NDC, 512], F8, tag=f"xs{s}{t}", name=f"xs{s}{t}")
                for s in range(NIC) for t in ("h", "l")
            }
            w8 = {
                k: persist.tile([128, NDC, E], F8, tag=f"w{k[0]}{k[1]}", name=f"w{k[0]}{k[1]}")
                for k in wd
            }
            wo16 = persist.tile([128, 2, D], BF16, tag="wo16", name="wo16")
            dm16 = persist.tile([128, NJ, 128], BF16, tag="dm16", name="dm16")
            mc16 = persist.tile([128, NJ, 3], BF16, tag="mc16", name="mc16")
            mxf = persist.tile([128, NJ, 2], F32, tag="mxf", name="mxf")
            ident = persist.tile([128, 128], BF16, tag="ident", name="ident")
            wtmp = persist.tile([128, 512], BF16, tag="wtmp", name="wtmp")
            q16 = [persist.tile([128, L], BF16, tag=f"q16_{ec}", name=f"q16_{ec}")
                   for ec in range(2)]
            k16 = [persist.tile([128, NLK], BF16, tag=f"k16_{ec}", name=f"k16_{ec}")
                   for ec in range(2)]
            u16 = [persist.tile([128, E], BF16, tag=f"u16_{lc}", name=f"u16_{lc}")
                   for lc in range(NLC)]
            vF8 = [persist.tile([128, E], BF16, tag=f"vF_{jc}", name=f"vF_{jc}")
                   for jc in range(NJ)]
            vP8 = [persist.tile([128, E], BF16, tag=f"vP_{jc}", name=f"vP_{jc}")
                   for jc in range(NJ)]
            g16 = [persist.tile([128, E], BF16, tag=f"g_{lc}", name=f"g_{lc}")
                   for lc in range(NLC)]
            gT16 = {(ec, lc): persist.tile([128, 128], BF16, tag=f"gt{ec}_{lc}", name=f"gt{ec}_{lc}")
                    for ec in range(2) for lc in range(NLC)}
            avs = persist.tile([128, 1024], F32, tag="avs", name="avs")
            rec16 = [persist.tile([128, 16], F32, tag=f"rec{p}", name=f"rec{p}")
                     for p in range(2)]

            # -------- emission helpers --------
            def dma_x(si, which=("h", "l")):
                s0 = si * 512
                for t in which:
                    nc.sync.dma_start(out=xs[(si, t)], in_=xd[t][:, :, s0 : s0 + 512])

            def proj_mms(p, w, lhs_of, rhs_of):
                """3-term hi/lo DR accumulation into psum slice p[:, 0:w]."""
                terms = (("h", "h"), ("h", "l"), ("l", "h"))
                n = NDC // 2
                first = True
                for (tx, tw) in terms:
                    for t in range(n):
                        nc.tensor.matmul(
                            p[:, 0:w],
                            lhs_of(tx, tw, t),
                            rhs_of(tx, tw, t),
                            start=first,
                            stop=(tx, tw) == ("l", "h") and t == n - 1,
                            perf_mode=DR,
                        )
                        first = False

            def proj_qk(pool, nm, ec, c0, w):
                """q16/k16[ec][:, c0:c0+w] = (x @ (16W).T)/16 in [feat, tok]."""
                p = pool.tile([128, 512], F32, tag="pp", name="pp")
                si, o = c0 // 512, c0 % 512
                proj_mms(
                    p, w,
                    lambda tx, tw, t: w8[(nm, tw)][:, 2 * t : 2 * t + 2, ec * 128 : (ec + 1) * 128],
                    lambda tx, tw, t: xs[(si, tx)][:, 2 * t : 2 * t + 2, o : o + w],
                )
                dest = q16 if nm == "q" else k16
                with nc.allow_low_precision(reason="bf16 store"):
                    nc.vector.tensor_scalar_mul(
                        dest[ec][:, c0 : c0 + w], p[:, 0:w], 1.0 / 16.0
                    )

            def proj_v(pool, h, jc):
                hsl = slice(64 * h, 64 * h + 64)
                si, o = (jc * 128) // 512, (jc * 128) % 512
                p = pool.tile([128, 512], F32, tag="pp", name="pp")
                proj_mms(
                    p, 64,
                    lambda tx, tw, t: xs[(si, tx)][:, 2 * t : 2 * t + 2, o : o + 128],
                    lambda tx, tw, t: w8[("v", tw)][:, 2 * t : 2 * t + 2, hsl],
                )
                with nc.allow_low_precision(reason="bf16 store"):
                    nc.vector.tensor_scalar_mul(
                        vF8[jc][:, hsl], p[:, 0:64], mxf[:, jc, 0:1]
                    )
                    nc.vector.tensor_scalar_mul(
                        vP8[jc][:, hsl], p[:, 0:64], mxf[:, jc, 1:2]
                    )

            def proj_u(pool, h, lc):
                hsl = slice(64 * h, 64 * h + 64)
                si, o = (lc * 128) // 512, (lc * 128) % 512
                p = pool.tile([128, 512], F32, tag="pp", name="pp")
                proj_mms(
                    p, 64,
                    lambda tx, tw, t: xs[(si, tx)][:, 2 * t : 2 * t + 2, o : o + 128],
                    lambda tx, tw, t: w8[("u", tw)][:, 2 * t : 2 * t + 2, hsl],
                )
                with nc.allow_low_precision(reason="bf16 store"):
                    nc.vector.tensor_scalar_mul(
                        u16[lc][:, hsl], p[:, 0:64], 1.0 / 16.0
                    )

            def scores_exp(scp, h, jc, half):
                """e tile [128 keys, 1024 queries] = exp(S/8) for (h, jc, half).
                Also precomputes the diag-masked eD tile when this (jc, half)
                contains the true-diagonal block."""
                ec, hh = h // 2, h % 2
                jsl = slice(jc * 128, (jc + 1) * 128)
                e8 = e8p.tile([128, 1024], BF16, tag="e8", name="e8")
                sc = scp.tile([128, 1024], F32, tag="sc", name="sc")
                for q in range(2):
                    q0 = half * 1024 + q * 512
                    nc.tensor.matmul(
                        sc[:, q * 512 : (q + 1) * 512],
                        k16[ec][64 * hh : 64 * hh + 64, jsl],
                        q16[ec][64 * hh : 64 * hh + 64, q0 : q0 + 512],
                        start=True, stop=True,
                    )
                with nc.allow_low_precision(reason="bf16 exp"):
                    nc.scalar.activation(e8, sc, EXP, scale=0.125)
                eD = None
                if half * 8 <= jc < half * 8 + 8:
                    eD = eDp.tile([128, 128], BF16, tag="eD", name="eD")
                    loc = jc * 128 - half * 1024
                    with nc.allow_low_precision(reason="mask mul"):
                        nc.gpsimd.tensor_mul(
                            eD, e8[:, loc : loc + 128], dm16[:, jc, :]
                        )
                return e8, eD

            def av_half(av, rs, h, jc, half, e8, eD):
                hsl = slice(64 * h, 64 * h + 64)
                base = half * 8
                for lc in range(base, base + 8):
                    loc = lc * 128 - half * 1024
                    if jc == lc:
                        lhsT, vt, mcol = eD, vF8, 2
                    elif jc < lc:
                        lhsT, vt, mcol = e8[:, loc : loc + 128], vF8, 0
                    else:
                        lhsT, vt, mcol = e8[:, loc : loc + 128], vP8, 1
                    nc.tensor.matmul(
                        av[:, lc * 64 : (lc + 1) * 64],
                        lhsT, vt[jc][:, hsl],
                        start=(jc == 0 and lc == base),
                        stop=(jc == NJ - 1 and lc == base + 7),
                    )
                    nc.tensor.matmul(
                        rs[:, (h % 2) * 16 + lc : (h % 2) * 16 + lc + 1],
                        lhsT, mc16[:, jc, mcol : mcol + 1],
                        start=(jc == 0 and half == 0 and lc == 0),
                        stop=(jc == NJ - 1 and lc == NLC - 1),
                    )

            def head_att(scp, projp, av, rs, h, pre=(), hooks=None):
                hooks = hooks or {}
                pend = []
                ui = 0
                for half in range(2):
                    for jc in range(NJ):
                        for f in hooks.get(ui, ()):
                            f()
                        e, eD = scores_exp(scp, h, jc, half)
                        if half == 0:
                            proj_v(projp, h, jc)
                        if ui == 0:
                            for f in pre:
                                f()
                        if len(pend) >= 3:
                            av_half(av, rs, h, *pend.pop(0))
                        pend.append((jc, half, e, eD))
                        ui += 1
                for item in pend:
                    av_half(av, rs, h, *item)

            def gate(av, rs, h):
                p = h % 2
                with nc.allow_low_precision(reason="gate"):
                    nc.vector.reciprocal(rec16[p], rs[:, p * 16 : (p + 1) * 16])
                    nc.vector.tensor_copy(avs, av)
                    for lc in range(NLC):
                        nc.vector.scalar_tensor_tensor(
                            g16[lc][:, 64 * h : 64 * h + 64],
                            avs[:, lc * 64 : (lc + 1) * 64],
                            rec16[p][:, lc : lc + 1],
                            u16[lc][:, 64 * h : 64 * h + 64],
                            MULT, MULT,
                        )

            def transposes_dma(ec):
                for lc in range(NLC):
                    nc.sync.dma_start_transpose(
                        gT16[(ec, lc)],
                        g16[lc][:, ec * 128 : (ec + 1) * 128],
                    )

            wo_alt = [0]
            osb_cur = [None]

            def wo_step(wop, ec, lc, fc, tail=False):
                """one W_o matmul + copy; every 8th step fires the quad DMA."""
                q, s = lc // 4, lc % 4
                if osb_cur[0] is None:
                    osb_cur[0] = osb.tile([128, 4, 1024], BF16, tag="osb", name="osb")
                o = osb_cur[0]
                p = wop.tile([128, 512], F32, tag="pp", name="pp")
                nc.tensor.matmul(
                    p,
                    gT16[(ec, lc)],
                    wo16[:, ec, fc * 512 : (fc + 1) * 512],
                    start=True, stop=True,
                )
                wo_alt[0] += 1
                with nc.allow_low_precision(reason="bf16 out"):
                    if tail and wo_alt[0] % 2 == 0:
                        nc.scalar.activation(
                            o[:, s, fc * 512 : (fc + 1) * 512], p, COPY
                        )
                    else:
                        nc.vector.tensor_copy(
                            o[:, s, fc * 512 : (fc + 1) * 512], p
                        )
                if s == 3 and fc == 1:
                    nc.sync.dma_start(
                        out=outr[ec][:, 4 * q : 4 * q + 4, :], in_=o
                    )
                    osb_cur[0] = None

            NU = 2 * NJ  # units per head

            def spread(jobs, lo, hi):
                """jobs: list of (cost, fn); place by cumulative cost."""
                hooks = {}
                total = sum(c for c, _ in jobs) or 1
                acc = 0
                for c, job in jobs:
                    hooks.setdefault(lo + (acc * (hi - lo)) // total, []).append(job)
                    acc += c
                return hooks

            with tc.tile_pool(name="av", bufs=1, space="PSUM") as avp, \
                 tc.tile_pool(name="rs", bufs=1, space="PSUM") as rsp:
                av = avp.tile([128, 1024], F32, tag="av", name="av")
                rs = rsp.tile([128, 32], F32, tag="rs", name="rs")

                # -------- phase 1: warmup, DMAs, h0, QK proj, U(h0) --------
                with tc.tile_pool(name="pp", bufs=3, space="PSUM") as pp, \
                     tc.tile_pool(name="sc1", bufs=1, space="PSUM") as sc1:
                    # PE warm-up chain during the initial DMA window
                    nc.vector.memset(wtmp, 0.0)
                    wp = pp.tile([128, 512], F32, tag="pp", name="pp")
                    for i in range(5):
                        nc.tensor.matmul(
                            wp, wtmp[:, 0:128], wtmp,
                            start=(i == 0), stop=(i == 4),
                        )

                    # input DMAs (x on SP queue, weights/masks on ACT queue)
                    dma_x(0, ("h",))
                    nc.scalar.dma_start(out=w8[("k", "h")], in_=wd[("k", "h")])
                    nc.scalar.dma_start(out=w8[("q", "h")], in_=wd[("q", "h")])
                    dma_x(1, ("h",))
                    dma_x(0, ("l",))
                    nc.scalar.dma_start(out=w8[("k", "l")], in_=wd[("k", "l")])
                    nc.scalar.dma_start(out=w8[("q", "l")], in_=wd[("q", "l")])
                    dma_x(1, ("l",))
                    for t in ("h", "l"):
                        nc.scalar.dma_start(out=w8[("v", t)], in_=wd[("v", t)])
                    for t in ("h", "l"):
                        nc.scalar.dma_start(out=w8[("u", t)], in_=wd[("u", t)])
                    nc.scalar.dma_start(out=dm16, in_=dm16d)
                    nc.scalar.dma_start(out=mc16, in_=mc16d)
                    nc.scalar.dma_start(out=mxf, in_=mxfd)
                    nc.scalar.dma_start(out=wo16, in_=wo16d)
                    nc.scalar.dma_start(out=ident, in_=identd)

                    proj_qk(pp, "k", 0, 0, 512)
                    proj_qk(pp, "q", 0, 0, 512)
                    proj_qk(pp, "q", 0, 512, 512)

                    jobs0 = []
                    jobs0.append((1, lambda: dma_x(2)))
                    for (c0, w) in kspans[1:2]:
                        jobs0.append((3, lambda c0=c0, w=w: proj_qk(pp, "k", 0, c0, w)))
                    jobs0.append((3, lambda: proj_qk(pp, "q", 0, 1024, 512)))
                    jobs0.append((1, lambda: dma_x(3)))
                    for (c0, w) in kspans[2:]:
                        jobs0.append((3, lambda c0=c0, w=w: proj_qk(pp, "k", 0, c0, w)))
                    jobs0.append((3, lambda: proj_qk(pp, "q", 0, 1536, 512)))
                    for lc in range(NLC):
                        jobs0.append((1, lambda lc=lc: proj_u(pp, 0, lc)))
                    for (c0, w) in kspans:
                        jobs0.append((3, lambda c0=c0, w=w: proj_qk(pp, "k", 1, c0, w)))
                    for ic in range(NIC):
                        jobs0.append((3, lambda ic=ic: proj_qk(pp, "q", 1, ic * 512, 512)))
                    head_att(sc1, pp, av, rs, 0, hooks=spread(jobs0, 1, NU))

                # -------- phase 2: h1-h3, ec0 wo --------
                with tc.tile_pool(name="sc2", bufs=2, space="PSUM") as sc2, \
                     tc.tile_pool(name="wop", bufs=1, space="PSUM") as wop:
                    jobs1 = [(1, lambda lc=lc: proj_u(wop, 1, lc)) for lc in range(NLC)]
                    head_att(sc2, wop, av, rs, 1,
                             pre=[lambda: gate(av, rs, 0)],
                             hooks=spread(jobs1, 1, NU))

                    jobs2 = [(1, lambda lc=lc: proj_u(wop, 2, lc)) for lc in range(NLC)]
                    jobs2 += [(1, lambda lc=lc, fc=fc: wo_step(wop, 0, lc, fc))
                              for lc in range(8) for fc in range(2)]
                    head_att(sc2, wop, av, rs, 2,
                             pre=[lambda: gate(av, rs, 1), lambda: transposes_dma(0)],
                             hooks=spread(jobs2, 1, NU))

                    jobs3 = [(1, lambda lc=lc: proj_u(wop, 3, lc)) for lc in range(NLC)]
                    jobs3 += [(1, lambda lc=lc, fc=fc: wo_step(wop, 0, lc, fc))
                              for lc in range(8, NLC) for fc in range(2)]
                    head_att(sc2, wop, av, rs, 3,
                             pre=[lambda: gate(av, rs, 2)],
                             hooks=spread(jobs3, 1, NU))
                    gate(av, rs, 3)

            # -------- phase 3: tail (av/rs closed): PE transposes + ec1 wo --------
            with tc.tile_pool(name="wo2", bufs=3, space="PSUM") as wo2, \
                 tc.tile_pool(name="tp", bufs=2, space="PSUM") as tpp:
                def tail_tp(lc):
                    t = tpp.tile([128, 128], BF16, tag="tp", name="tp")
                    nc.tensor.transpose(t, g16[lc][:, 128:256], ident)
                    with nc.allow_low_precision(reason="bf16 transpose"):
                        if lc % 2 == 0:
                            nc.vector.tensor_copy(gT16[(1, lc)], t)
                        else:
                            nc.scalar.activation(gT16[(1, lc)], t, COPY)

                def tail_wo(lc):
                    q, s = lc // 4, lc % 4
                    if osb_cur[0] is None:
                        osb_cur[0] = osb.tile([128, 4, 1024], BF16, tag="osb", name="osb")
                    o = osb_cur[0]
                    p = wo2.tile([128, 1024], F32, tag="wq", name="wq")
                    for fc in range(2):
                        nc.tensor.matmul(
                            p[:, fc * 512 : (fc + 1) * 512],
                            gT16[(1, lc)],
                            wo16[:, 1, fc * 512 : (fc + 1) * 512],
                            start=True, stop=True,
                        )
                    with nc.allow_low_precision(reason="bf16 out"):
                        if lc % 2 == 0:
                            nc.scalar.activation(o[:, s, :], p, COPY)
                        else:
                            nc.vector.tensor_copy(o[:, s, :], p)
                    if s == 3:
                        nc.sync.dma_start(
                            out=outr[1][:, 4 * q : 4 * q + 4, :], in_=o
                        )
                        osb_cur[0] = None

                tail_tp(0)
                tail_tp(1)
                for lc in range(NLC):
                    if lc + 2 < NLC:
                        tail_tp(lc + 2)
                    tail_wo(lc)

    nc.compile()
    return nc


def _hilo(a):
    f8 = ml_dtypes.float8_e4m3
    hi = a.astype(f8)
    lo = (a - hi.astype(np.float32)).astype(f8)
    return hi, lo


def _host_inputs(NJ, x, token_types, seq_lens, W_q, W_k, W_v, W_u, W_o):
    x = np.asarray(x, dtype=np.float32)
    token_types = np.asarray(token_types)
    seq_lens = np.asarray(seq_lens)
    W = {
        "q": np.asarray(W_q, dtype=np.float32),
        "k": np.asarray(W_k, dtype=np.float32),
        "v": np.asarray(W_v, dtype=np.float32),
        "u": np.asarray(W_u, dtype=np.float32),
    }
    W_o = np.asarray(W_o, dtype=np.float32)
    bf = ml_dtypes.bfloat16

    per_batch = []
    for b in range(B):
        xt = np.ascontiguousarray(x[b].T.reshape(NDC, 128, L).transpose(1, 0, 2))
        xh, xl = _hilo(xt)
        prompt = np.asarray(token_types[b] < 3)
        valid = np.arange(L) < int(seq_lens[b])
        dm16 = np.zeros((128, NJ, 128), bf)
        mc16 = np.zeros((128, NJ, 3), bf)
        mxf = np.zeros((128, NJ, 2), np.float32)
        for jc in range(NJ):
            j = np.arange(jc * 128, (jc + 1) * 128)
            i = j  # true-diagonal block
            allow = valid[j][:, None] & (prompt[j][:, None] | (j[:, None] <= i[None, :]))
            dm16[:, jc, :] = allow.astype(np.float32)
            mF = valid[j].astype(np.float32)
            mP = (valid[j] & prompt[j]).astype(np.float32)
            mc16[:, jc, 0] = mF
            mc16[:, jc, 1] = mP
            mc16[:, jc, 2] = 1.0
            mxf[:, jc, 0] = mF / 16.0
            mxf[:, jc, 1] = mP / 16.0
        per_batch.append((xh, xl, dm16, mc16, mxf))

    ident = np.eye(128, dtype=bf)
    in_maps = []
    for c in range(8):
        b, gi = c // 4, c % 4
        e0 = E * gi
        xh, xl, dm16, mc16, mxf = per_batch[b]
        m = {"x_h": xh, "x_l": xl, "dm16": dm16, "mc16": mc16, "mxf": mxf,
             "ident": ident}
        for nm in ("q", "k", "v", "u"):
            wt = np.ascontiguousarray(
                (W[nm][e0 : e0 + E] * 16.0).T.reshape(NDC, 128, E).transpose(1, 0, 2)
            )
            wh, wl = _hilo(wt)
            m[f"w{nm}_h"] = wh
            m[f"w{nm}_l"] = wl
        m["wo16"] = np.ascontiguousarray(
            W_o[:, e0 : e0 + E].T.reshape(2, 128, D).transpose(1, 0, 2)
        ).astype(bf)
        in_maps.append(m)
    return in_maps


def kernel(x, token_types, seq_lens, W_q, W_k, W_v, W_u, W_o, **_run_kwargs):
    seq = np.asarray(seq_lens)
    NJ = int(np.ceil(seq.max() / 128.0))
    NJ = max(1, min(NLC, NJ))
    if ("nc", NJ) not in _cache:
        _cache[("nc", NJ)] = _build(NJ)
    nc = _cache[("nc", NJ)]
    in_maps = _host_inputs(NJ, x, token_types, seq_lens, W_q, W_k, W_v, W_u, W_o)
    try:
        res = run_bass_kernel_spmd(nc, in_maps, list(range(8)), **_run_kwargs)
    except Exception as ex:  # transient NRT device wedge: retry once
        if "UNRECOVERABLE" not in str(ex) and "UNAVAILABLE" not in str(ex):
            raise
        res = run_bass_kernel_spmd(nc, in_maps, list(range(8)), **_run_kwargs)
    _cache["last_result"] = res
    _cache["nc"] = nc  # for test.py TimelineSim
    full = np.zeros((B, L, D), np.float64)
    for c in range(8):
        r = res.results[c]
        full[c // 4] += r["out0"].astype(np.float64) + r["out1"].astype(np.float64)
    return full.astype(np.float32)
